# revision 1
# baseline (speedup 1.0000x reference)
"""GCN (2-layer, symmetric-norm message passing) on 8 Trainium2 NeuronCores.

Contract: kernel(**inputs) takes the FULL inputs (x [50000,4,300] f32,
edge_index [2,250000] i32, W1/b1/W2/b2) and returns the FULL output
[50000,300] f32.

Strategy (per sharding hint): shard destination nodes across the 8 cores
(6250 each), replicate the small weights, partition edges by destination so
scatter-adds are core-local, and AllGather the pre-scaled source features
between layers.  The scatter-add itself is computed on the PE array as a
sequence of 0/1-indicator matmuls over 128-edge chunks (edges sorted by
destination on the host), with the per-row gather done by indirect DMA.
"""

import math

import numpy as np

import concourse.bacc as bacc
import concourse.bass as bass
import concourse.tile as tile
from concourse import bass_utils, mybir
from concourse.bass import IndirectOffsetOnAxis
from concourse.masks import make_identity

F32 = mybir.dt.float32
BF16 = mybir.dt.bfloat16
I32 = mybir.dt.int32
P = 128

N_CORES = 8


def _cdiv(a, b):
    return (a + b - 1) // b


# ---------------------------------------------------------------- host prep


def prep_inputs(x, edge_index, W1, b1, W2, b2, n_cores=N_CORES):
    """Shard + preprocess the full inputs into per-core in_maps.

    Returns (in_maps, meta) where meta carries the dims needed to build the
    device program.
    """
    N, T, C = x.shape
    assert N % n_cores == 0
    NPC = N // n_cores
    NBLK = _cdiv(NPC, P)

    row = np.asarray(edge_index[0], dtype=np.int64)
    col = np.asarray(edge_index[1], dtype=np.int64)

    # symmetric sqrt-degree norm; degree on source (row), +1 for self loops
    deg = (np.bincount(row, minlength=N) + 1).astype(np.float32)
    dis = (deg.astype(np.float32) ** -0.5).astype(np.float32)

    core_of = col // NPC

    # first pass: per-core per-block edge counts -> global CPB
    per_core = []
    max_blk = 0
    for c in range(n_cores):
        m = core_of == c
        r = row[m]
        d = col[m] - c * NPC
        order = np.argsort(d, kind="stable")
        r = r[order]
        d = d[order]
        cnt = np.bincount(d // P, minlength=NBLK)
        max_blk = max(max_blk, int(cnt.max()) if len(cnt) else 0)
        per_core.append((r, d, cnt))
    CPB = max(1, _cdiv(max_blk, P))

    # replicated tensors
    CC = [(c0, min(P, C - c0)) for c0 in range(0, C, P)]
    KC = len(CC)
    import ml_dtypes

    w1c = np.zeros((KC, P, C), ml_dtypes.bfloat16)
    w2c = np.zeros((KC, P, C), ml_dtypes.bfloat16)
    for k, (c0, cs) in enumerate(CC):
        w1c[k, :cs, :] = (W1.T[c0 : c0 + cs, :] / np.float32(T)).astype(np.float32)
        w2c[k, :cs, :] = W2.T[c0 : c0 + cs, :].astype(np.float32)
    b1t = np.broadcast_to(np.asarray(b1, np.float32), (P, C)).copy()
    b2t = np.broadcast_to(np.asarray(b2, np.float32), (P, C)).copy()
    iota = np.broadcast_to(np.arange(P, dtype=np.float32), (P, P)).copy()

    in_maps = []
    for c in range(n_cores):
        r, d, cnt = per_core[c]
        starts = np.concatenate([[0], np.cumsum(cnt)])
        idxt = np.zeros((P, NBLK * CPB), np.int32)
        dlt = np.full((P, NBLK * CPB), -1.0, np.float32)
        for blk in range(NBLK):
            s, e = int(starts[blk]), int(starts[blk + 1])
            n = e - s
            pad = CPB * P
            rb = np.zeros(pad, np.int64)
            rb[:n] = r[s:e]
            db = np.full(pad, -1.0, np.float32)
            db[:n] = (d[s:e] - blk * P).astype(np.float32)
            idxt[:, blk * CPB : (blk + 1) * CPB] = (
                rb.reshape(CPB, P).T.astype(np.int32)
            )
            dlt[:, blk * CPB : (blk + 1) * CPB] = db.reshape(CPB, P).T

        dis_c = dis[c * NPC : (c + 1) * NPC]
        dist = np.zeros((P, NBLK), np.float32)
        dist.reshape(-1)[: 0] = 0  # noop, keep shape
        flat = np.zeros(NBLK * P, np.float32)
        flat[:NPC] = dis_c
        dist[:, :] = flat.reshape(NBLK, P).T

        in_maps.append(
            {
                "xs": np.ascontiguousarray(x[c * NPC : (c + 1) * NPC]).astype(
                    np.float32
                ),
                "w1c": w1c,
                "w2c": w2c,
                "b1t": b1t,
                "b2t": b2t,
                "iot": iota,
                "dist": dist,
                "idxt": idxt,
                "dlt": dlt,
            }
        )

    meta = dict(N=N, T=T, C=C, NPC=NPC, NBLK=NBLK, CPB=CPB, CC=CC, n_cores=n_cores)
    return in_maps, meta


# ------------------------------------------------------------- device build


def build_nc(meta):
    N = meta["N"]
    T = meta["T"]
    C = meta["C"]
    NPC = meta["NPC"]
    NBLK = meta["NBLK"]
    CPB = meta["CPB"]
    CC = meta["CC"]
    KC = len(CC)
    n_cores = meta["n_cores"]
    rg = [list(range(n_cores))]

    nc = bacc.Bacc(
        "TRN2", target_bir_lowering=False, debug=False, num_devices=n_cores
    )

    xs = nc.dram_tensor("xs", [NPC, T, C], F32, kind="ExternalInput")
    w1c = nc.dram_tensor("w1c", [KC, P, C], BF16, kind="ExternalInput")
    w2c = nc.dram_tensor("w2c", [KC, P, C], BF16, kind="ExternalInput")
    b1t = nc.dram_tensor("b1t", [P, C], F32, kind="ExternalInput")
    b2t = nc.dram_tensor("b2t", [P, C], F32, kind="ExternalInput")
    iot = nc.dram_tensor("iot", [P, P], F32, kind="ExternalInput")
    dist = nc.dram_tensor("dist", [P, NBLK], F32, kind="ExternalInput")
    idxt = nc.dram_tensor("idxt", [P, NBLK * CPB], I32, kind="ExternalInput")
    dlt = nc.dram_tensor("dlt", [P, NBLK * CPB], F32, kind="ExternalInput")
    out_ext = nc.dram_tensor("out", [NPC, C], F32, kind="ExternalOutput")

    ACT = mybir.ActivationFunctionType

    with tile.TileContext(nc) as tc:
        with (
            tc.tile_pool(name="dramp", bufs=1, space="DRAM") as dramp,
            tc.tile_pool(name="singles", bufs=1) as singles,
            tc.tile_pool(name="work", bufs=3) as wp,
            tc.tile_pool(name="msgs", bufs=12) as mp,
            tc.tile_pool(name="psA", bufs=1, space="PSUM") as psA,
            tc.tile_pool(name="psT", bufs=2, space="PSUM") as psT,
            tc.tile_pool(name="psB", bufs=3, space="PSUM") as psB,
            tc.tile_pool(name="psC", bufs=2, space="PSUM") as psC,
        ):
            agin1 = dramp.tile([NPC, C], BF16, name="agin1")
            hp1f = dramp.tile([N, C], BF16, addr_space="Shared", name="hp1f")
            agin2 = dramp.tile([NPC, C], BF16, name="agin2")
            hp2f = dramp.tile([N, C], BF16, addr_space="Shared", name="hp2f")

            # constants / tables in SBUF
            ident = singles.tile([P, P], BF16, name="ident")
            make_identity(nc, ident[:])
            w1sb = singles.tile([P, KC, C], BF16, name="w1sb")
            w2sb = singles.tile([P, KC, C], BF16, name="w2sb")
            for k in range(KC):
                nc.sync.dma_start(out=w1sb[:, k, :], in_=w1c[k])
                nc.sync.dma_start(out=w2sb[:, k, :], in_=w2c[k])
            b1sb = singles.tile([P, C], F32, name="b1sb")
            nc.sync.dma_start(out=b1sb[:], in_=b1t[:])
            b2sb = singles.tile([P, C], F32, name="b2sb")
            nc.sync.dma_start(out=b2sb[:], in_=b2t[:])
            iosb = singles.tile([P, P], F32, name="iosb")
            nc.sync.dma_start(out=iosb[:], in_=iot[:])
            dissb = singles.tile([P, NBLK], F32, name="dissb")
            nc.sync.dma_start(out=dissb[:], in_=dist[:])
            idxsb = singles.tile([P, NBLK * CPB], I32, name="idxsb")
            nc.sync.dma_start(out=idxsb[:], in_=idxt[:])
            dlsb = singles.tile([P, NBLK * CPB], F32, name="dlsb")
            nc.sync.dma_start(out=dlsb[:], in_=dlt[:])

            # resident self-term tiles: hps = dis * hp = dis^2 * h
            hps1 = singles.tile([P, NBLK, C], F32, name="hps1")
            hps2 = singles.tile([P, NBLK, C], F32, name="hps2")
            if NPC % P != 0:
                # zero once so partial-block tail rows stay zero
                nc.vector.memset(hps1[:], 0.0)
                nc.vector.memset(hps2[:], 0.0)
            def ag_full(agin, hpf):
                nc.gpsimd.collective_compute(
                    "AllGather",
                    mybir.AluOpType.bypass,
                    replica_groups=rg,
                    ins=[agin.opt()],
                    outs=[hpf.opt()],
                )

            # ---------------- stage A: h = mean_t(x) @ W1.T + b1, prescale
            for b in range(NBLK):
                Pb = min(P, NPC - b * P)
                dcol = dissb[:Pb, b : b + 1]
                xt = wp.tile([P, T, C], F32, tag="xt")
                nc.sync.dma_start(out=xt[:Pb], in_=xs[b * P : b * P + Pb])
                s0 = wp.tile([P, C], F32, tag="s0")
                s1 = wp.tile([P, C], F32, tag="s1")
                xm = wp.tile([P, C], BF16, tag="xm")
                nc.vector.tensor_add(out=s0[:Pb], in0=xt[:Pb, 0], in1=xt[:Pb, 1])
                nc.vector.tensor_add(out=s1[:Pb], in0=xt[:Pb, 2], in1=xt[:Pb, 3])
                nc.vector.tensor_add(out=xm[:Pb], in0=s0[:Pb], in1=s1[:Pb])
                hpp = psA.tile([P, C], F32, tag="hpp")
                for k, (c0, cs) in enumerate(CC):
                    ptr = psT.tile([P, P], BF16, tag="ptr")
                    nc.tensor.transpose(
                        out=ptr[:cs, :Pb],
                        in_=xm[:Pb, c0 : c0 + cs],
                        identity=ident[:Pb, :Pb],
                    )
                    xT = wp.tile([P, P], BF16, tag="xT")
                    nc.scalar.copy(out=xT[:cs, :Pb], in_=ptr[:cs, :Pb])
                    nc.tensor.matmul(
                        out=hpp[:Pb],
                        lhsT=xT[:cs, :Pb],
                        rhs=w1sb[:cs, k, :],
                        start=(k == 0),
                        stop=(k == KC - 1),
                    )
                th = wp.tile([P, C], F32, tag="th")
                nc.vector.tensor_add(out=th[:Pb], in0=hpp[:Pb], in1=b1sb[:Pb])
                hp_t = wp.tile([P, C], BF16, tag="hp")
                nc.scalar.activation(out=hp_t[:Pb], in_=th[:Pb], func=ACT.Copy, scale=dcol)
                nc.sync.dma_start(out=agin1[b * P : b * P + Pb], in_=hp_t[:Pb])
                nc.scalar.activation(
                    out=hps1[:Pb, b, :], in_=hp_t[:Pb], func=ACT.Copy, scale=dcol
                )
                if b == NBLK - 1:
                    ag_full(agin1, hp1f)


            # ------------- prop core: gather + indicator matmuls -> psum
            def prop_psum(b, src_full, pool):
                pp = pool.tile([P, C], F32, tag="pp")
                for ch in range(CPB):
                    j = b * CPB + ch
                    msg = mp.tile([P, C], BF16, tag="msg")
                    nc.gpsimd.indirect_dma_start(
                        out=msg[:],
                        out_offset=None,
                        in_=src_full[:],
                        in_offset=IndirectOffsetOnAxis(
                            ap=idxsb[:, j : j + 1], axis=0
                        ),
                    )
                    ind = wp.tile([P, P], BF16, tag="ind")
                    nc.vector.tensor_tensor(
                        out=ind[:],
                        in0=iosb[:],
                        in1=dlsb[:, j : j + 1].to_broadcast([P, P]),
                        op=mybir.AluOpType.is_equal,
                    )
                    nc.tensor.matmul(
                        out=pp[:],
                        lhsT=ind[:],
                        rhs=msg[:],
                        start=(ch == 0),
                        stop=(ch == CPB - 1),
                    )
                return pp

            # ---------------- layer 1 prop + layer 2 linear (fused per block)
            for b in range(NBLK):
                Pb = min(P, NPC - b * P)
                dcol = dissb[:, b : b + 1]
                pp = prop_psum(b, hp1f, psB)
                t1 = wp.tile([P, C], F32, tag="t1")
                nc.vector.scalar_tensor_tensor(
                    out=t1[:],
                    in0=pp[:],
                    scalar=dcol,
                    in1=hps1[:, b, :],
                    op0=mybir.AluOpType.mult,
                    op1=mybir.AluOpType.add,
                )
                h1 = wp.tile([P, C], BF16, tag="h1")
                nc.vector.scalar_tensor_tensor(
                    out=h1[:],
                    in0=t1[:],
                    scalar=0.01,
                    in1=t1[:],
                    op0=mybir.AluOpType.mult,
                    op1=mybir.AluOpType.max,
                )
                h2p = psC.tile([P, C], F32, tag="h2p")
                for k, (c0, cs) in enumerate(CC):
                    ptr2 = psT.tile([P, P], BF16, tag="ptr")
                    nc.tensor.transpose(
                        out=ptr2[:cs, :], in_=h1[:, c0 : c0 + cs], identity=ident[:]
                    )
                    hT = wp.tile([P, P], BF16, tag="hT")
                    nc.scalar.copy(out=hT[:cs, :], in_=ptr2[:cs, :])
                    nc.tensor.matmul(
                        out=h2p[:],
                        lhsT=hT[:cs, :],
                        rhs=w2sb[:cs, k, :],
                        start=(k == 0),
                        stop=(k == KC - 1),
                    )
                t2 = wp.tile([P, C], F32, tag="t2")
                nc.vector.tensor_add(out=t2[:], in0=h2p[:], in1=b2sb[:])
                hp2_t = wp.tile([P, C], BF16, tag="hp2")
                nc.scalar.activation(
                    out=hp2_t[:Pb], in_=t2[:Pb], func=ACT.Copy, scale=dissb[:Pb, b : b + 1]
                )
                nc.sync.dma_start(out=agin2[b * P : b * P + Pb], in_=hp2_t[:Pb])
                nc.scalar.activation(
                    out=hps2[:Pb, b, :],
                    in_=hp2_t[:Pb],
                    func=ACT.Copy,
                    scale=dissb[:Pb, b : b + 1],
                )
                if b == NBLK - 1:
                    ag_full(agin2, hp2f)


            # ---------------- layer 2 prop -> output
            for b in range(NBLK):
                Pb = min(P, NPC - b * P)
                dcol = dissb[:, b : b + 1]
                pp = prop_psum(b, hp2f, psB)
                ot = wp.tile([P, C], F32, tag="ot")
                nc.vector.scalar_tensor_tensor(
                    out=ot[:],
                    in0=pp[:],
                    scalar=dcol,
                    in1=hps2[:, b, :],
                    op0=mybir.AluOpType.mult,
                    op1=mybir.AluOpType.add,
                )
                nc.sync.dma_start(out=out_ext[b * P : b * P + Pb], in_=ot[:Pb])

    nc.compile()
    return nc


# ------------------------------------------------------------------ runner

_CACHE = {}


def run(x, edge_index, W1, b1, W2, b2, n_cores=N_CORES, trace=False):
    in_maps, meta = prep_inputs(x, edge_index, W1, b1, W2, b2, n_cores)
    key = (meta["N"], meta["T"], meta["C"], meta["CPB"], n_cores)
    if key not in _CACHE:
        _CACHE[key] = build_nc(meta)
    nc = _CACHE[key]
    res = bass_utils.run_bass_kernel_spmd(
        nc, in_maps, core_ids=list(range(n_cores)), trace=trace
    )
    NPC = meta["NPC"]
    outs = [np.asarray(res.results[c]["out"]) for c in range(n_cores)]
    full = np.concatenate(outs, axis=0).astype(np.float32)
    return full, res


def kernel(x, edge_index, W1, b1, W2, b2):
    x = np.asarray(x)
    edge_index = np.asarray(edge_index)
    full, _ = run(
        np.asarray(x, np.float32),
        edge_index,
        np.asarray(W1, np.float32),
        np.asarray(b1, np.float32),
        np.asarray(W2, np.float32),
        np.asarray(b2, np.float32),
    )
    return full



# revision 3
# speedup vs baseline: 1.1416x; 1.1416x over previous
"""GCN (2-layer, symmetric-norm message passing) on 8 Trainium2 NeuronCores.

Contract: kernel(**inputs) takes the FULL inputs (x [50000,4,300] f32,
edge_index [2,250000] i32, W1/b1/W2/b2) and returns the FULL output
[50000,300] f32.

Strategy (per sharding hint): shard destination nodes across the 8 cores
(6250 each), replicate the small weights, partition edges by destination so
scatter-adds are core-local, and AllGather the pre-scaled source features
between layers.  The scatter-add itself is computed on the PE array as a
sequence of 0/1-indicator matmuls over 128-edge chunks (edges sorted by
destination on the host), with the per-row gather done by indirect DMA.
"""

import math

import numpy as np

import concourse.bacc as bacc
import concourse.bass as bass
import concourse.tile as tile
from concourse import bass_utils, mybir
from concourse.bass import IndirectOffsetOnAxis
from concourse.masks import make_identity

F32 = mybir.dt.float32
BF16 = mybir.dt.bfloat16
I32 = mybir.dt.int32
P = 128

N_CORES = 8


def _cdiv(a, b):
    return (a + b - 1) // b


# ---------------------------------------------------------------- host prep


def prep_inputs(x, edge_index, W1, b1, W2, b2, n_cores=N_CORES):
    """Shard + preprocess the full inputs into per-core in_maps.

    Returns (in_maps, meta) where meta carries the dims needed to build the
    device program.
    """
    N, T, C = x.shape
    assert N % n_cores == 0
    NPC = N // n_cores
    NBLK = _cdiv(NPC, P)

    row = np.asarray(edge_index[0], dtype=np.int64)
    col = np.asarray(edge_index[1], dtype=np.int64)

    # symmetric sqrt-degree norm; degree on source (row), +1 for self loops
    deg = (np.bincount(row, minlength=N) + 1).astype(np.float32)
    dis = (deg.astype(np.float32) ** -0.5).astype(np.float32)

    core_of = col // NPC

    # first pass: per-core per-block edge counts -> global CPB
    per_core = []
    max_blk = 0
    for c in range(n_cores):
        m = core_of == c
        r = row[m]
        d = col[m] - c * NPC
        order = np.argsort(d, kind="stable")
        r = r[order]
        d = d[order]
        cnt = np.bincount(d // P, minlength=NBLK)
        max_blk = max(max_blk, int(cnt.max()) if len(cnt) else 0)
        per_core.append((r, d, cnt))
    CPB = max(1, _cdiv(max_blk, P))

    # replicated tensors
    CC = [(c0, min(P, C - c0)) for c0 in range(0, C, P)]
    KC = len(CC)
    import ml_dtypes

    w1c = np.zeros((KC, P, C), ml_dtypes.bfloat16)
    w2c = np.zeros((KC, P, C), ml_dtypes.bfloat16)
    for k, (c0, cs) in enumerate(CC):
        w1c[k, :cs, :] = (W1.T[c0 : c0 + cs, :] / np.float32(T)).astype(np.float32)
        w2c[k, :cs, :] = W2.T[c0 : c0 + cs, :].astype(np.float32)
    b1t = np.broadcast_to(np.asarray(b1, np.float32), (P, C)).copy()
    b2t = np.broadcast_to(np.asarray(b2, np.float32), (P, C)).copy()
    iota = np.broadcast_to(np.arange(P, dtype=np.float32), (P, P)).copy()

    in_maps = []
    for c in range(n_cores):
        r, d, cnt = per_core[c]
        starts = np.concatenate([[0], np.cumsum(cnt)])
        idxt = np.zeros((P, NBLK * CPB), np.int32)
        dlt = np.full((P, NBLK * CPB), -1.0, np.float32)
        for blk in range(NBLK):
            s, e = int(starts[blk]), int(starts[blk + 1])
            n = e - s
            pad = CPB * P
            rb = np.zeros(pad, np.int64)
            rb[:n] = r[s:e]
            db = np.full(pad, -1.0, np.float32)
            db[:n] = (d[s:e] - blk * P).astype(np.float32)
            idxt[:, blk * CPB : (blk + 1) * CPB] = (
                rb.reshape(CPB, P).T.astype(np.int32)
            )
            dlt[:, blk * CPB : (blk + 1) * CPB] = db.reshape(CPB, P).T

        dis_c = dis[c * NPC : (c + 1) * NPC]
        dist = np.zeros((P, NBLK), np.float32)
        flat = np.zeros(NBLK * P, np.float32)
        flat[:NPC] = dis_c
        dist[:, :] = flat.reshape(NBLK, P).T

        in_maps.append(
            {
                "xs": np.ascontiguousarray(x[c * NPC : (c + 1) * NPC]).astype(
                    np.float32
                ),
                "w1c": w1c,
                "w2c": w2c,
                "b1t": b1t,
                "b2t": b2t,
                "iot": iota,
                "dist": dist,
                "idxt": idxt,
                "dlt": dlt,
            }
        )

    meta = dict(N=N, T=T, C=C, NPC=NPC, NBLK=NBLK, CPB=CPB, CC=CC, n_cores=n_cores)
    return in_maps, meta


# ------------------------------------------------------------- device build


def build_nc(meta):
    N = meta["N"]
    T = meta["T"]
    C = meta["C"]
    NPC = meta["NPC"]
    NBLK = meta["NBLK"]
    CPB = meta["CPB"]
    CC = meta["CC"]
    KC = len(CC)
    n_cores = meta["n_cores"]
    rg = [list(range(n_cores))]

    nc = bacc.Bacc(
        "TRN2", target_bir_lowering=False, debug=False, num_devices=n_cores
    )

    xs = nc.dram_tensor("xs", [NPC, T, C], F32, kind="ExternalInput")
    w1c = nc.dram_tensor("w1c", [KC, P, C], BF16, kind="ExternalInput")
    w2c = nc.dram_tensor("w2c", [KC, P, C], BF16, kind="ExternalInput")
    b1t = nc.dram_tensor("b1t", [P, C], F32, kind="ExternalInput")
    b2t = nc.dram_tensor("b2t", [P, C], F32, kind="ExternalInput")
    iot = nc.dram_tensor("iot", [P, P], F32, kind="ExternalInput")
    dist = nc.dram_tensor("dist", [P, NBLK], F32, kind="ExternalInput")
    idxt = nc.dram_tensor("idxt", [P, NBLK * CPB], I32, kind="ExternalInput")
    dlt = nc.dram_tensor("dlt", [P, NBLK * CPB], F32, kind="ExternalInput")
    out_ext = nc.dram_tensor("out", [NPC, C], F32, kind="ExternalOutput")

    ACT = mybir.ActivationFunctionType

    with tile.TileContext(nc) as tc:
        with (
            tc.tile_pool(name="dramp", bufs=1, space="DRAM") as dramp,
            tc.tile_pool(name="singles", bufs=1) as singles,
            tc.tile_pool(name="work", bufs=3) as wp,
            tc.tile_pool(name="msgs", bufs=12) as mp,
            tc.tile_pool(name="psA", bufs=1, space="PSUM") as psA,
            tc.tile_pool(name="psT", bufs=2, space="PSUM") as psT,
            tc.tile_pool(name="psB", bufs=3, space="PSUM") as psB,
            tc.tile_pool(name="psC", bufs=2, space="PSUM") as psC,
        ):
            agin1 = dramp.tile([NPC, C], BF16, name="agin1")
            hp1f = dramp.tile([N, C], BF16, addr_space="Shared", name="hp1f")
            agin2 = dramp.tile([NPC, C], BF16, name="agin2")
            hp2f = dramp.tile([N, C], BF16, addr_space="Shared", name="hp2f")

            # constants / tables in SBUF
            ident = singles.tile([P, P], BF16, name="ident")
            make_identity(nc, ident[:])
            w1sb = singles.tile([P, KC, C], BF16, name="w1sb")
            w2sb = singles.tile([P, KC, C], BF16, name="w2sb")
            for k in range(KC):
                nc.sync.dma_start(out=w1sb[:, k, :], in_=w1c[k])
                nc.sync.dma_start(out=w2sb[:, k, :], in_=w2c[k])
            b1sb = singles.tile([P, C], F32, name="b1sb")
            nc.sync.dma_start(out=b1sb[:], in_=b1t[:])
            b2sb = singles.tile([P, C], F32, name="b2sb")
            nc.sync.dma_start(out=b2sb[:], in_=b2t[:])
            iosb = singles.tile([P, P], F32, name="iosb")
            nc.sync.dma_start(out=iosb[:], in_=iot[:])
            dissb = singles.tile([P, NBLK], F32, name="dissb")
            nc.sync.dma_start(out=dissb[:], in_=dist[:])
            idxsb = singles.tile([P, NBLK * CPB], I32, name="idxsb")
            nc.sync.dma_start(out=idxsb[:], in_=idxt[:])
            dlsb = singles.tile([P, NBLK * CPB], F32, name="dlsb")
            nc.sync.dma_start(out=dlsb[:], in_=dlt[:])

            # resident self-term tiles: hps = dis * hp = dis^2 * h
            hps1 = singles.tile([P, NBLK, C], F32, name="hps1")
            hps2 = singles.tile([P, NBLK, C], F32, name="hps2")
            if NPC % P != 0:
                # zero once so partial-block tail rows stay zero
                nc.vector.memset(hps1[:], 0.0)
                nc.vector.memset(hps2[:], 0.0)

            def ag_full(agin, hpf):
                nc.gpsimd.collective_compute(
                    "AllGather",
                    mybir.AluOpType.bypass,
                    replica_groups=rg,
                    ins=[agin.opt()],
                    outs=[hpf.opt()],
                )

            # ---------------- stage A: h = mean_t(x) @ W1.T + b1, prescale
            for b in range(NBLK):
                Pb = min(P, NPC - b * P)
                dcol = dissb[:Pb, b : b + 1]
                xt = wp.tile([P, T, C], F32, tag="xt")
                nc.sync.dma_start(out=xt[:Pb], in_=xs[b * P : b * P + Pb])
                s0 = wp.tile([P, C], F32, tag="s0")
                s1 = wp.tile([P, C], F32, tag="s1")
                xm = wp.tile([P, C], BF16, tag="xm")
                nc.vector.tensor_add(out=s0[:Pb], in0=xt[:Pb, 0], in1=xt[:Pb, 1])
                nc.vector.tensor_add(out=s1[:Pb], in0=xt[:Pb, 2], in1=xt[:Pb, 3])
                nc.vector.tensor_add(out=xm[:Pb], in0=s0[:Pb], in1=s1[:Pb])
                hpp = psA.tile([P, C], F32, tag="hpp")
                for k, (c0, cs) in enumerate(CC):
                    ptr = psT.tile([P, P], BF16, tag="ptr")
                    nc.tensor.transpose(
                        out=ptr[:cs, :Pb],
                        in_=xm[:Pb, c0 : c0 + cs],
                        identity=ident[:Pb, :Pb],
                    )
                    xT = wp.tile([P, P], BF16, tag="xT")
                    nc.scalar.copy(out=xT[:cs, :Pb], in_=ptr[:cs, :Pb])
                    nc.tensor.matmul(
                        out=hpp[:Pb],
                        lhsT=xT[:cs, :Pb],
                        rhs=w1sb[:cs, k, :],
                        start=(k == 0),
                        stop=(k == KC - 1),
                    )
                th = wp.tile([P, C], F32, tag="th")
                nc.vector.tensor_add(out=th[:Pb], in0=hpp[:Pb], in1=b1sb[:Pb])
                hp_t = wp.tile([P, C], BF16, tag="hp")
                nc.scalar.activation(out=hp_t[:Pb], in_=th[:Pb], func=ACT.Copy, scale=dcol)
                nc.sync.dma_start(out=agin1[b * P : b * P + Pb], in_=hp_t[:Pb])
                nc.scalar.activation(
                    out=hps1[:Pb, b, :], in_=hp_t[:Pb], func=ACT.Copy, scale=dcol
                )
                if b == NBLK - 1:
                    ag_full(agin1, hp1f)

            # ------------- prop core: gather + indicator matmuls -> psum
            def prop_psum(b, src_full, pool):
                pp = pool.tile([P, C], F32, tag="pp")
                for ch in range(CPB):
                    j = b * CPB + ch
                    msg = mp.tile([P, C], BF16, tag="msg")
                    nc.gpsimd.indirect_dma_start(
                        out=msg[:],
                        out_offset=None,
                        in_=src_full[:],
                        in_offset=IndirectOffsetOnAxis(
                            ap=idxsb[:, j : j + 1], axis=0
                        ),
                    )
                    ind = wp.tile([P, P], BF16, tag="ind")
                    nc.vector.tensor_tensor(
                        out=ind[:],
                        in0=iosb[:],
                        in1=dlsb[:, j : j + 1].to_broadcast([P, P]),
                        op=mybir.AluOpType.is_equal,
                    )
                    nc.tensor.matmul(
                        out=pp[:],
                        lhsT=ind[:],
                        rhs=msg[:],
                        start=(ch == 0),
                        stop=(ch == CPB - 1),
                    )
                return pp

            # ---------------- layer 1 prop + layer 2 linear (fused per block)
            for b in range(NBLK):
                Pb = min(P, NPC - b * P)
                dcol = dissb[:, b : b + 1]
                pp = prop_psum(b, hp1f, psB)
                t1 = wp.tile([P, C], F32, tag="t1")
                nc.vector.scalar_tensor_tensor(
                    out=t1[:],
                    in0=pp[:],
                    scalar=dcol,
                    in1=hps1[:, b, :],
                    op0=mybir.AluOpType.mult,
                    op1=mybir.AluOpType.add,
                )
                h1 = wp.tile([P, C], BF16, tag="h1")
                nc.vector.scalar_tensor_tensor(
                    out=h1[:],
                    in0=t1[:],
                    scalar=0.01,
                    in1=t1[:],
                    op0=mybir.AluOpType.mult,
                    op1=mybir.AluOpType.max,
                )
                h2p = psC.tile([P, C], F32, tag="h2p")
                for k, (c0, cs) in enumerate(CC):
                    ptr2 = psT.tile([P, P], BF16, tag="ptr")
                    nc.tensor.transpose(
                        out=ptr2[:cs, :], in_=h1[:, c0 : c0 + cs], identity=ident[:]
                    )
                    hT = wp.tile([P, P], BF16, tag="hT")
                    nc.scalar.copy(out=hT[:cs, :], in_=ptr2[:cs, :])
                    nc.tensor.matmul(
                        out=h2p[:],
                        lhsT=hT[:cs, :],
                        rhs=w2sb[:cs, k, :],
                        start=(k == 0),
                        stop=(k == KC - 1),
                    )
                t2 = wp.tile([P, C], F32, tag="t2")
                nc.vector.tensor_add(out=t2[:], in0=h2p[:], in1=b2sb[:])
                hp2_t = wp.tile([P, C], BF16, tag="hp2")
                nc.scalar.activation(
                    out=hp2_t[:Pb], in_=t2[:Pb], func=ACT.Copy, scale=dissb[:Pb, b : b + 1]
                )
                nc.sync.dma_start(out=agin2[b * P : b * P + Pb], in_=hp2_t[:Pb])
                nc.scalar.activation(
                    out=hps2[:Pb, b, :],
                    in_=hp2_t[:Pb],
                    func=ACT.Copy,
                    scale=dissb[:Pb, b : b + 1],
                )
                if b == NBLK - 1:
                    ag_full(agin2, hp2f)

            # ---------------- layer 2 prop -> output
            for b in range(NBLK):
                Pb = min(P, NPC - b * P)
                dcol = dissb[:, b : b + 1]
                pp = prop_psum(b, hp2f, psB)
                ot = wp.tile([P, C], F32, tag="ot")
                nc.vector.scalar_tensor_tensor(
                    out=ot[:],
                    in0=pp[:],
                    scalar=dcol,
                    in1=hps2[:, b, :],
                    op0=mybir.AluOpType.mult,
                    op1=mybir.AluOpType.add,
                )
                nc.sync.dma_start(out=out_ext[b * P : b * P + Pb], in_=ot[:Pb])

    nc.compile()
    return nc


# ------------------------------------------------------------------ runner

_CACHE = {}


def run(x, edge_index, W1, b1, W2, b2, n_cores=N_CORES, trace=False):
    in_maps, meta = prep_inputs(x, edge_index, W1, b1, W2, b2, n_cores)
    key = (meta["N"], meta["T"], meta["C"], meta["CPB"], n_cores)
    if key not in _CACHE:
        _CACHE[key] = build_nc(meta)
    nc = _CACHE[key]
    res = bass_utils.run_bass_kernel_spmd(
        nc, in_maps, core_ids=list(range(n_cores)), trace=trace
    )
    NPC = meta["NPC"]
    outs = [np.asarray(res.results[c]["out"]) for c in range(n_cores)]
    full = np.concatenate(outs, axis=0).astype(np.float32)
    return full, res


def kernel(x, edge_index, W1, b1, W2, b2):
    x = np.asarray(x)
    edge_index = np.asarray(edge_index)
    full, _ = run(
        np.asarray(x, np.float32),
        edge_index,
        np.asarray(W1, np.float32),
        np.asarray(b1, np.float32),
        np.asarray(W2, np.float32),
        np.asarray(b2, np.float32),
    )
    return full


# revision 4
# speedup vs baseline: 1.2038x; 1.0545x over previous
"""GCN (2-layer, symmetric-norm message passing) on 8 Trainium2 NeuronCores.

v3: destination-sharded GCN with
  - per-chunk [P,1] indirect-DMA gathers (the only HW-correct gather form),
  - group-pooled edge chunks (7 dst blocks per group, chunks packed across
    block boundaries) to minimize the ~1us/call SWDGE fixed cost,
  - the feature table / AllGather split into two segments (blocks 0..23 /
    24..48 of each core) so each AllGather overlaps neighboring compute,
  - host-pretransposed x so stage A needs no on-chip transposes,
  - biases added via rank-1 ones-matmuls into PSUM,
  - batched indicator construction (one 3D is_equal per group).
"""

import numpy as np

import concourse.bacc as bacc
import concourse.bass as bass
import concourse.tile as tile
from concourse import bass_utils, mybir
from concourse.bass import IndirectOffsetOnAxis
from concourse.masks import make_identity

F32 = mybir.dt.float32
BF16 = mybir.dt.bfloat16
I32 = mybir.dt.int32
P = 128

N_CORES = 8
GB = 7          # dst blocks per gather group
ASPLIT = 24     # table segment split (blocks 0..ASPLIT-1 -> A, rest -> B)


def _cdiv(a, b):
    return (a + b - 1) // b


# ---------------------------------------------------------------- host prep


def prep_inputs(x, edge_index, W1, b1, W2, b2, n_cores=N_CORES):
    import ml_dtypes

    N, T, C = x.shape
    NPC = N // n_cores
    NBLK = _cdiv(NPC, P)
    NG = NBLK // GB
    NA = ASPLIT * P            # rows per core in segment A
    NB = NPC - NA              # rows per core in segment B

    row = np.asarray(edge_index[0], dtype=np.int64)
    col = np.asarray(edge_index[1], dtype=np.int64)

    deg = (np.bincount(row, minlength=N) + 1).astype(np.float32)
    dis = (deg.astype(np.float32) ** -0.5).astype(np.float32)

    # table-row mapping for source node g: half, row
    src_r = row // NPC
    src_i = row % NPC
    in_A = src_i < NA
    tab_row = np.where(in_A, src_r * NA + src_i, src_r * NB + (src_i - NA))

    core_of = col // NPC

    # per-core, per-group, per-half edge lists (sorted by dst)
    # lists[c][g][h] = (tab_rows, dst_local_in_block, dst_block)
    lists = [[[None, None] for _ in range(NG)] for _ in range(n_cores)]
    for c in range(n_cores):
        m = core_of == c
        tr = tab_row[m]
        ia = in_A[m]
        d = col[m] - c * NPC
        order = np.argsort(d, kind="stable")
        tr, ia, d = tr[order], ia[order], d[order]
        blk = d // P
        grp = blk // GB
        for g in range(NG):
            gm = grp == g
            for h in range(2):
                hm = gm & (ia == (h == 0))
                lists[c][g][h] = (
                    tr[hm].astype(np.int64),
                    (d[hm] - blk[hm] * P).astype(np.int64),
                    blk[hm].astype(np.int64),
                )

    # chunk counts per (g, half): max over cores
    NCH = np.zeros((NG, 2), np.int64)
    for g in range(NG):
        for h in range(2):
            NCH[g, h] = max(
                _cdiv(len(lists[c][g][h][0]), P) for c in range(n_cores)
            )
    NCHA = int(NCH[:, 0].max())
    NCHB = int(NCH[:, 1].max())
    NCHG = NCHA + NCHB  # idx cols per group (uniform layout)

    # indicator columns: union over cores of blocks spanned by each chunk
    # ind_cols[g] = list of (h, jloc, b); mm_of_block[b] = ordered list of
    # (ind_local, h, jloc)
    ind_cols = []
    mm_of_block = [[] for _ in range(NBLK)]
    for g in range(NG):
        cols = []
        for h in range(2):
            for j in range(int(NCH[g, h])):
                spanned = set()
                for c in range(n_cores):
                    blks = lists[c][g][h][2][j * P : (j + 1) * P]
                    spanned.update(np.unique(blks).tolist())
                for b in sorted(spanned):
                    cols.append((h, j, int(b)))
        ind_cols.append(cols)
        for i, (h, j, b) in enumerate(cols):
            mm_of_block[b].append((i, h, j))
    NIND = max(len(c) for c in ind_cols)

    # replicated weights; stage-A contraction runs over (c,t) jointly so the
    # T-mean folds into the matmul: 1200 rows split into KC1 chunks of CS1.
    CS1 = 120
    KC1 = (C * T) // CS1
    w1rows = np.repeat(W1.T, T, axis=0) / np.float32(T)  # row c*T+t -> W1.T[c]/T
    w1ct = np.zeros((KC1, CS1, C), ml_dtypes.bfloat16)
    for k in range(KC1):
        w1ct[k] = w1rows[k * CS1 : (k + 1) * CS1, :].astype(np.float32)
    CC2 = [(c0, min(P, C - c0)) for c0 in range(0, C, P)]
    KC2 = len(CC2)
    w2c = np.zeros((KC2, P, C), ml_dtypes.bfloat16)
    for k, (c0, cs) in enumerate(CC2):
        w2c[k, :cs, :] = W2.T[c0 : c0 + cs, :].astype(np.float32)
    b1r = np.asarray(b1, np.float32).reshape(1, C).astype(ml_dtypes.bfloat16)
    b2r = np.asarray(b2, np.float32).reshape(1, C).astype(ml_dtypes.bfloat16)
    iot = np.broadcast_to(np.arange(P, dtype=np.float32), (P, P)).astype(
        ml_dtypes.bfloat16
    )

    in_maps = []
    for c in range(n_cores):
        idxt = np.zeros((P, NG * NCHG), np.int32)
        dlt = np.full((P, NG * NIND), -1.0, np.float32)
        for g in range(NG):
            for h in range(2):
                tr, dl, blk = lists[c][g][h]
                n = len(tr)
                nch = int(NCH[g, h])
                pad = nch * P
                trp = np.zeros(pad, np.int64)
                trp[:n] = tr
                base = g * NCHG + (0 if h == 0 else NCHA)
                idxt[:, base : base + nch] = trp.reshape(nch, P).T.astype(
                    np.int32
                )
            for i, (h, j, b) in enumerate(ind_cols[g]):
                tr, dl, blk = lists[c][g][h]
                n = len(tr)
                sl = np.full(P, -1.0, np.float32)
                s0 = j * P
                seg = slice(s0, min(s0 + P, n))
                nseg = seg.stop - seg.start
                if nseg > 0:
                    dseg = dl[seg]
                    bseg = blk[seg]
                    vals = np.where(bseg == b, dseg.astype(np.float32), -1.0)
                    sl[:nseg] = vals
                dlt[:, g * NIND + i] = sl

        dis_c = dis[c * NPC : (c + 1) * NPC]
        flat = np.zeros(NBLK * P, np.float32)
        flat[:NPC] = dis_c
        dist = flat.reshape(NBLK, P).T.copy()

        xc = np.asarray(x[c * NPC : (c + 1) * NPC], np.float32)
        xpad = np.zeros((NBLK * P, T, C), np.float32)
        xpad[:NPC] = xc
        # [NBLK, P(n), T, C] -> rows j=c*T+t chunked [NBLK, KC1, CS1, P] ->
        # per-block tile [CS1, KC1, P]
        xr = xpad.reshape(NBLK, P, T, C).transpose(0, 3, 2, 1)  # [NBLK, C, T, P]
        xr = xr.reshape(NBLK, KC1, CS1, P).transpose(0, 2, 1, 3)
        xsd = np.ascontiguousarray(xr).reshape(NBLK * CS1, KC1 * P)

        in_maps.append(
            {
                "xsd": xsd,
                "w1ct": w1ct,
                "w2c": w2c,
                "b1r": b1r,
                "b2r": b2r,
                "iot": iot,
                "dist": dist,
                "idxt": idxt,
                "dlt": dlt.astype(ml_dtypes.bfloat16),
            }
        )

    meta = dict(
        N=N, T=T, C=C, NPC=NPC, NBLK=NBLK, NG=NG, NA=NA, NB=NB,
        NCH=NCH.tolist(), NCHA=NCHA, NCHB=NCHB, NCHG=NCHG, NIND=NIND,
        ind_cols=ind_cols, mm_of_block=mm_of_block, CS1=CS1, KC1=KC1,
        CC2=CC2, n_cores=n_cores,
    )
    return in_maps, meta


# ------------------------------------------------------------- device build


def build_nc(meta):
    N = meta["N"]
    T = meta["T"]
    C = meta["C"]
    NPC = meta["NPC"]
    NBLK = meta["NBLK"]
    NG = meta["NG"]
    NA = meta["NA"]
    NB = meta["NB"]
    NCH = meta["NCH"]
    NCHA = meta["NCHA"]
    NCHB = meta["NCHB"]
    NCHG = meta["NCHG"]
    NIND = meta["NIND"]
    ind_cols = meta["ind_cols"]
    mm_of_block = meta["mm_of_block"]
    CS1 = meta["CS1"]
    KC1 = meta["KC1"]
    CC2 = meta["CC2"]
    KC2 = len(CC2)
    n_cores = meta["n_cores"]
    rg = [list(range(n_cores))]

    nc = bacc.Bacc(
        "TRN2", target_bir_lowering=False, debug=False, num_devices=n_cores
    )

    xsd = nc.dram_tensor("xsd", [NBLK * CS1, KC1 * P], F32, kind="ExternalInput")
    w1ct = nc.dram_tensor("w1ct", [KC1, CS1, C], BF16, kind="ExternalInput")
    w2c = nc.dram_tensor("w2c", [KC2, P, C], BF16, kind="ExternalInput")
    b1r = nc.dram_tensor("b1r", [1, C], BF16, kind="ExternalInput")
    b2r = nc.dram_tensor("b2r", [1, C], BF16, kind="ExternalInput")
    iot = nc.dram_tensor("iot", [P, P], BF16, kind="ExternalInput")
    dist = nc.dram_tensor("dist", [P, NBLK], F32, kind="ExternalInput")
    idxt = nc.dram_tensor("idxt", [P, NG * NCHG], I32, kind="ExternalInput")
    dlt = nc.dram_tensor("dlt", [P, NG * NIND], BF16, kind="ExternalInput")
    out_ext = nc.dram_tensor("out", [NPC, C], F32, kind="ExternalOutput")

    ACT = mybir.ActivationFunctionType

    with tile.TileContext(nc) as tc:
        with (
            tc.tile_pool(name="dramp", bufs=1, space="DRAM") as dramp,
            tc.tile_pool(name="singles", bufs=1) as singles,
            tc.tile_pool(name="xp", bufs=3) as xp,
            tc.tile_pool(name="sp", bufs=2) as sp,
            tc.tile_pool(name="wp", bufs=4) as wp,
            tc.tile_pool(name="hp", bufs=4) as hpp_pool,
            tc.tile_pool(name="msA", bufs=4) as mpA,
            tc.tile_pool(name="msB", bufs=2) as mpB,
            tc.tile_pool(name="inds", bufs=2) as ip,
            tc.tile_pool(name="psA", bufs=2, space="PSUM") as psA,
            tc.tile_pool(name="psT", bufs=2, space="PSUM") as psT,
            tc.tile_pool(name="psB", bufs=2, space="PSUM") as psB,
            tc.tile_pool(name="psC", bufs=2, space="PSUM") as psC,
        ):
            ag1A = dramp.tile([NA, C], BF16, name="ag1A")
            ag1B = dramp.tile([NB, C], BF16, name="ag1B")
            hp1A = dramp.tile([n_cores * NA, C], BF16, addr_space="Shared", name="hp1A")
            hp1B = dramp.tile([n_cores * NB, C], BF16, addr_space="Shared", name="hp1B")
            ag2A = dramp.tile([NA, C], BF16, name="ag2A")
            ag2B = dramp.tile([NB, C], BF16, name="ag2B")
            hp2A = dramp.tile([n_cores * NA, C], BF16, addr_space="Shared", name="hp2A")
            hp2B = dramp.tile([n_cores * NB, C], BF16, addr_space="Shared", name="hp2B")

            ident = singles.tile([P, P], BF16, name="ident")
            make_identity(nc, ident[:])
            ones1 = singles.tile([1, P], BF16, name="ones1")
            nc.vector.memset(ones1[:], 1.0)
            w1sb = singles.tile([CS1, KC1, C], BF16, name="w1sb")
            for k in range(KC1):
                nc.sync.dma_start(out=w1sb[:, k, :], in_=w1ct[k])
            w2sb = singles.tile([P, KC2, C], BF16, name="w2sb")
            for k in range(KC2):
                nc.sync.dma_start(out=w2sb[:, k, :], in_=w2c[k])
            b1sb = singles.tile([1, C], BF16, name="b1sb")
            nc.sync.dma_start(out=b1sb[:], in_=b1r[:])
            b2sb = singles.tile([1, C], BF16, name="b2sb")
            nc.sync.dma_start(out=b2sb[:], in_=b2r[:])
            iosb = singles.tile([P, 1, P], BF16, name="iosb")
            nc.sync.dma_start(out=iosb[:, 0, :], in_=iot[:])
            dissb = singles.tile([P, NBLK], F32, name="dissb")
            nc.sync.dma_start(out=dissb[:], in_=dist[:])
            idxsb = singles.tile([P, NG * NCHG], I32, name="idxsb")
            nc.sync.dma_start(out=idxsb[:], in_=idxt[:])
            dlsb = singles.tile([P, NG * NIND], BF16, name="dlsb")
            nc.sync.dma_start(out=dlsb[:], in_=dlt[:])

            hps1 = singles.tile([P, NBLK, C], BF16, name="hps1")
            hps2 = singles.tile([P, NBLK, C], BF16, name="hps2")

            def ag(agin, hpf):
                nc.gpsimd.collective_compute(
                    "AllGather",
                    mybir.AluOpType.bypass,
                    replica_groups=rg,
                    ins=[agin.opt()],
                    outs=[hpf.opt()],
                )

            # ---------------- stage A
            for b in range(NBLK):
                Pb = min(P, NPC - b * P)
                dcol = dissb[:, b : b + 1]
                xt = xp.tile([CS1, KC1, P], F32, tag="xt")
                nc.sync.dma_start(out=xt[:], in_=xsd[b * CS1 : (b + 1) * CS1, :])
                xm = sp.tile([CS1, KC1, P], BF16, tag="xm")
                nc.vector.tensor_copy(out=xm[:], in_=xt[:, :, :])
                hpt = psA.tile([P, C], F32, tag="hpt")
                for k in range(KC1):
                    nc.tensor.matmul(
                        out=hpt[:],
                        lhsT=xm[:, k, :],
                        rhs=w1sb[:, k, :],
                        start=(k == 0),
                        stop=False,
                    )
                nc.tensor.matmul(
                    out=hpt[:], lhsT=ones1[:], rhs=b1sb[:], start=False, stop=True
                )
                hp_t = wp.tile([P, C], BF16, tag="hp")
                nc.scalar.activation(out=hp_t[:], in_=hpt[:], func=ACT.Copy, scale=dcol)
                if b < ASPLIT:
                    nc.sync.dma_start(
                        out=ag1A[b * P : b * P + Pb], in_=hp_t[:Pb]
                    )
                else:
                    r0 = (b - ASPLIT) * P
                    nc.sync.dma_start(out=ag1B[r0 : r0 + Pb], in_=hp_t[:Pb])
                nc.scalar.activation(
                    out=hps1[:, b, :], in_=hp_t[:], func=ACT.Copy, scale=dcol
                )
                if b == ASPLIT - 1:
                    with tc.high_priority():
                        ag(ag1A, hp1A)
                if b == NBLK - 1:
                    with tc.high_priority():
                        ag(ag1B, hp1B)

            # ------------- shared prop helpers
            def emit_gathers(g, h, srcA, srcB, pool):
                ncols = int(NCH[g][h])
                nmax = NCHA if h == 0 else NCHB
                msg = pool.tile([P, nmax, C], BF16, tag=f"m{h}")
                src = srcA if h == 0 else srcB
                base = g * NCHG + (0 if h == 0 else NCHA)
                for j in range(ncols):
                    nc.gpsimd.indirect_dma_start(
                        out=msg[:, j, :],
                        out_offset=None,
                        in_=src[:],
                        in_offset=IndirectOffsetOnAxis(
                            ap=idxsb[:, base + j : base + j + 1], axis=0
                        ),
                    )
                return msg

            def emit_ind(g):
                ind = ip.tile([P, NIND, P], BF16, tag="ind")
                nc.vector.tensor_tensor(
                    out=ind[:],
                    in0=iosb[:, :, :].to_broadcast([P, NIND, P]),
                    in1=dlsb[:, g * NIND : (g + 1) * NIND].to_broadcast(
                        [P, NIND, P]
                    ),
                    op=mybir.AluOpType.is_equal,
                )
                return ind

            def prop_block(b, msgA, msgB, ind):
                mm = mm_of_block[b]
                pp = psB.tile([P, C], F32, tag="pp")
                for i, (icol, h, j) in enumerate(mm):
                    msg = msgA if h == 0 else msgB
                    nc.tensor.matmul(
                        out=pp[:],
                        lhsT=ind[:, icol, :],
                        rhs=msg[:, j, :],
                        start=(i == 0),
                        stop=(i == len(mm) - 1),
                    )
                return pp

            # schedule tokens for a prop pass: stagger A-gathers one group
            # ahead of B-gathers so the pool queue rarely stalls on segment B
            def prop_pass(srcA, srcB, consume):
                msgsA = {}
                msgsB = {}
                inds = {}
                for gg in range(min(3, NG)):
                    msgsA[gg] = emit_gathers(gg, 0, srcA, srcB, mpA)
                    inds[gg] = emit_ind(gg)
                for g in range(NG):
                    msgsB[g] = emit_gathers(g, 1, srcA, srcB, mpB)
                    if g + 3 < NG:
                        msgsA[g + 3] = emit_gathers(g + 3, 0, srcA, srcB, mpA)
                        inds[g + 3] = emit_ind(g + 3)
                    for bl in range(GB):
                        b = g * GB + bl
                        consume(b, msgsA[g], msgsB[g], inds[g])
                    del msgsA[g], msgsB[g], inds[g]

            # ---------------- layer 1 prop + layer 2 linear
            def consume_l1(b, msgA, msgB, ind):
                Pb = min(P, NPC - b * P)
                dcol = dissb[:, b : b + 1]
                pp = prop_block(b, msgA, msgB, ind)
                t1 = hpp_pool.tile([P, C], F32, tag="t1")
                nc.vector.scalar_tensor_tensor(
                    out=t1[:],
                    in0=pp[:],
                    scalar=dcol,
                    in1=hps1[:, b, :],
                    op0=mybir.AluOpType.mult,
                    op1=mybir.AluOpType.add,
                )
                h1 = hpp_pool.tile([P, C], BF16, tag="h1")
                nc.vector.scalar_tensor_tensor(
                    out=h1[:],
                    in0=t1[:],
                    scalar=0.01,
                    in1=t1[:],
                    op0=mybir.AluOpType.mult,
                    op1=mybir.AluOpType.max,
                )
                h2p = psC.tile([P, C], F32, tag="h2p")
                for k, (c0, cs) in enumerate(CC2):
                    ptr2 = psT.tile([P, P], BF16, tag="ptr")
                    nc.tensor.transpose(
                        out=ptr2[:cs, :], in_=h1[:, c0 : c0 + cs], identity=ident[:]
                    )
                    hT = wp.tile([P, P], BF16, tag="hT")
                    nc.vector.tensor_copy(out=hT[:cs, :], in_=ptr2[:cs, :])
                    nc.tensor.matmul(
                        out=h2p[:],
                        lhsT=hT[:cs, :],
                        rhs=w2sb[:cs, k, :],
                        start=(k == 0),
                        stop=False,
                    )
                nc.tensor.matmul(
                    out=h2p[:], lhsT=ones1[:], rhs=b2sb[:], start=False, stop=True
                )
                hp2_t = wp.tile([P, C], BF16, tag="hp2")
                nc.scalar.activation(
                    out=hp2_t[:], in_=h2p[:], func=ACT.Copy, scale=dcol
                )
                if b < ASPLIT:
                    nc.sync.dma_start(out=ag2A[b * P : b * P + Pb], in_=hp2_t[:Pb])
                else:
                    r0 = (b - ASPLIT) * P
                    nc.sync.dma_start(out=ag2B[r0 : r0 + Pb], in_=hp2_t[:Pb])
                nc.scalar.activation(
                    out=hps2[:, b, :], in_=hp2_t[:], func=ACT.Copy, scale=dcol
                )
                # trigger the layer-2 AllGather segments as soon as their
                # inputs are complete (two blocks of slack for the epilogue)
                if b == ASPLIT + 1:
                    ag(ag2A, hp2A)
                if b == NBLK - 1:
                    ag(ag2B, hp2B)

            prop_pass(hp1A, hp1B, consume_l1)

            # ---------------- layer 2 prop -> output
            def consume_l2(b, msgA, msgB, ind):
                Pb = min(P, NPC - b * P)
                dcol = dissb[:, b : b + 1]
                pp = prop_block(b, msgA, msgB, ind)
                ot = hpp_pool.tile([P, C], F32, tag="ot")
                nc.vector.scalar_tensor_tensor(
                    out=ot[:],
                    in0=pp[:],
                    scalar=dcol,
                    in1=hps2[:, b, :],
                    op0=mybir.AluOpType.mult,
                    op1=mybir.AluOpType.add,
                )
                nc.sync.dma_start(out=out_ext[b * P : b * P + Pb], in_=ot[:Pb])

            prop_pass(hp2A, hp2B, consume_l2)

    nc.compile()
    return nc


# ------------------------------------------------------------------ runner

_CACHE = {}


def run(x, edge_index, W1, b1, W2, b2, n_cores=N_CORES, trace=False):
    in_maps, meta = prep_inputs(x, edge_index, W1, b1, W2, b2, n_cores)
    key = (meta["N"], meta["T"], meta["C"], meta["NCHA"], meta["NCHB"],
           meta["NIND"], n_cores)
    if key not in _CACHE:
        _CACHE[key] = build_nc(meta)
    nc = _CACHE[key]
    res = bass_utils.run_bass_kernel_spmd(
        nc, in_maps, core_ids=list(range(n_cores)), trace=trace
    )
    outs = [np.asarray(res.results[c]["out"]) for c in range(n_cores)]
    full = np.concatenate(outs, axis=0).astype(np.float32)
    return full, res


def kernel(x, edge_index, W1, b1, W2, b2):
    x = np.asarray(x)
    edge_index = np.asarray(edge_index)
    full, _ = run(
        np.asarray(x, np.float32),
        edge_index,
        np.asarray(W1, np.float32),
        np.asarray(b1, np.float32),
        np.asarray(W2, np.float32),
        np.asarray(b2, np.float32),
    )
    return full


# revision 5
# speedup vs baseline: 1.2116x; 1.0064x over previous
"""GCN (2-layer, symmetric-norm message passing) on 8 Trainium2 NeuronCores.

v3: destination-sharded GCN with
  - per-chunk [P,1] indirect-DMA gathers (the only HW-correct gather form),
  - group-pooled edge chunks (7 dst blocks per group, chunks packed across
    block boundaries) to minimize the ~1us/call SWDGE fixed cost,
  - the feature table / AllGather split into two segments (blocks 0..23 /
    24..48 of each core) so each AllGather overlaps neighboring compute,
  - host-pretransposed x so stage A needs no on-chip transposes,
  - biases added via rank-1 ones-matmuls into PSUM,
  - batched indicator construction (one 3D is_equal per group).
"""

import numpy as np

import concourse.bacc as bacc
import concourse.bass as bass
import concourse.tile as tile
from concourse import bass_utils, mybir
from concourse.bass import IndirectOffsetOnAxis
from concourse.masks import make_identity

F32 = mybir.dt.float32
BF16 = mybir.dt.bfloat16
I32 = mybir.dt.int32
P = 128

N_CORES = 8
GB = 7          # dst blocks per gather group
ASPLIT = 24     # table segment split (blocks 0..ASPLIT-1 -> A, rest -> B)


def _cdiv(a, b):
    return (a + b - 1) // b


# ---------------------------------------------------------------- host prep


def prep_inputs(x, edge_index, W1, b1, W2, b2, n_cores=N_CORES):
    import ml_dtypes

    N, T, C = x.shape
    NPC = N // n_cores
    NBLK = _cdiv(NPC, P)
    NG = NBLK // GB
    NA = ASPLIT * P            # rows per core in segment A
    NB = NPC - NA              # rows per core in segment B

    row = np.asarray(edge_index[0], dtype=np.int64)
    col = np.asarray(edge_index[1], dtype=np.int64)

    deg = (np.bincount(row, minlength=N) + 1).astype(np.float32)
    dis = (deg.astype(np.float32) ** -0.5).astype(np.float32)

    # table-row mapping for source node g: half, row
    src_r = row // NPC
    src_i = row % NPC
    in_A = src_i < NA
    tab_row = np.where(in_A, src_r * NA + src_i, src_r * NB + (src_i - NA))

    core_of = col // NPC

    # per-core, per-group, per-half edge lists (sorted by dst)
    # lists[c][g][h] = (tab_rows, dst_local_in_block, dst_block)
    lists = [[[None, None] for _ in range(NG)] for _ in range(n_cores)]
    for c in range(n_cores):
        m = core_of == c
        tr = tab_row[m]
        ia = in_A[m]
        d = col[m] - c * NPC
        order = np.argsort(d, kind="stable")
        tr, ia, d = tr[order], ia[order], d[order]
        blk = d // P
        grp = blk // GB
        for g in range(NG):
            gm = grp == g
            for h in range(2):
                hm = gm & (ia == (h == 0))
                lists[c][g][h] = (
                    tr[hm].astype(np.int64),
                    (d[hm] - blk[hm] * P).astype(np.int64),
                    blk[hm].astype(np.int64),
                )

    # chunk counts per (g, half): max over cores
    NCH = np.zeros((NG, 2), np.int64)
    for g in range(NG):
        for h in range(2):
            NCH[g, h] = max(
                _cdiv(len(lists[c][g][h][0]), P) for c in range(n_cores)
            )
    NCHA = int(NCH[:, 0].max())
    NCHB = int(NCH[:, 1].max())
    NCHG = NCHA + NCHB  # idx cols per group (uniform layout)

    # indicator columns: union over cores of blocks spanned by each chunk
    # ind_cols[g] = list of (h, jloc, b); mm_of_block[b] = ordered list of
    # (ind_local, h, jloc)
    ind_cols = []
    mm_of_block = [[] for _ in range(NBLK)]
    for g in range(NG):
        cols = []
        for h in range(2):
            for j in range(int(NCH[g, h])):
                spanned = set()
                for c in range(n_cores):
                    blks = lists[c][g][h][2][j * P : (j + 1) * P]
                    spanned.update(np.unique(blks).tolist())
                for b in sorted(spanned):
                    cols.append((h, j, int(b)))
        ind_cols.append(cols)
        for i, (h, j, b) in enumerate(cols):
            mm_of_block[b].append((i, h, j))
    NIND = max(len(c) for c in ind_cols)

    # replicated weights; stage-A contraction runs over (c,t) jointly so the
    # T-mean folds into the matmul: 1200 rows split into KC1 chunks of CS1.
    CS1 = 120
    KC1 = (C * T) // CS1
    w1rows = np.repeat(W1.T, T, axis=0) / np.float32(T)  # row c*T+t -> W1.T[c]/T
    w1ct = np.zeros((KC1, CS1, C), ml_dtypes.bfloat16)
    for k in range(KC1):
        w1ct[k] = w1rows[k * CS1 : (k + 1) * CS1, :].astype(np.float32)
    CC2 = [(c0, min(P, C - c0)) for c0 in range(0, C, P)]
    KC2 = len(CC2)
    w2c = np.zeros((KC2, P, C), ml_dtypes.bfloat16)
    for k, (c0, cs) in enumerate(CC2):
        w2c[k, :cs, :] = W2.T[c0 : c0 + cs, :].astype(np.float32)
    b1r = np.asarray(b1, np.float32).reshape(1, C).astype(ml_dtypes.bfloat16)
    b2r = np.asarray(b2, np.float32).reshape(1, C).astype(ml_dtypes.bfloat16)
    iot = np.broadcast_to(np.arange(P, dtype=np.float32), (P, P)).astype(
        ml_dtypes.bfloat16
    )

    in_maps = []
    for c in range(n_cores):
        idxt = np.zeros((P, NG * NCHG), np.int32)
        dlt = np.full((P, NG * NIND), -1.0, np.float32)
        for g in range(NG):
            for h in range(2):
                tr, dl, blk = lists[c][g][h]
                n = len(tr)
                nch = int(NCH[g, h])
                pad = nch * P
                trp = np.zeros(pad, np.int64)
                trp[:n] = tr
                base = g * NCHG + (0 if h == 0 else NCHA)
                idxt[:, base : base + nch] = trp.reshape(nch, P).T.astype(
                    np.int32
                )
            for i, (h, j, b) in enumerate(ind_cols[g]):
                tr, dl, blk = lists[c][g][h]
                n = len(tr)
                sl = np.full(P, -1.0, np.float32)
                s0 = j * P
                seg = slice(s0, min(s0 + P, n))
                nseg = seg.stop - seg.start
                if nseg > 0:
                    dseg = dl[seg]
                    bseg = blk[seg]
                    vals = np.where(bseg == b, dseg.astype(np.float32), -1.0)
                    sl[:nseg] = vals
                dlt[:, g * NIND + i] = sl

        dis_c = dis[c * NPC : (c + 1) * NPC]
        flat = np.zeros(NBLK * P, np.float32)
        flat[:NPC] = dis_c
        dist = flat.reshape(NBLK, P).T.copy()

        xc = np.asarray(x[c * NPC : (c + 1) * NPC], np.float32)
        xpad = np.zeros((NBLK * P, T, C), np.float32)
        xpad[:NPC] = xc
        # [NBLK, P(n), T, C] -> rows j=c*T+t chunked [NBLK, KC1, CS1, P] ->
        # per-block tile [CS1, KC1, P]
        xr = xpad.reshape(NBLK, P, T, C).transpose(0, 3, 2, 1)  # [NBLK, C, T, P]
        xr = xr.reshape(NBLK, KC1, CS1, P).transpose(0, 2, 1, 3)
        xsd = (
            np.ascontiguousarray(xr)
            .reshape(NBLK * CS1, KC1 * P)
            .astype(ml_dtypes.bfloat16)
        )

        in_maps.append(
            {
                "xsd": xsd,
                "w1ct": w1ct,
                "w2c": w2c,
                "b1r": b1r,
                "b2r": b2r,
                "iot": iot,
                "dist": dist,
                "idxt": idxt,
                "dlt": dlt.astype(ml_dtypes.bfloat16),
            }
        )

    meta = dict(
        N=N, T=T, C=C, NPC=NPC, NBLK=NBLK, NG=NG, NA=NA, NB=NB,
        NCH=NCH.tolist(), NCHA=NCHA, NCHB=NCHB, NCHG=NCHG, NIND=NIND,
        ind_cols=ind_cols, mm_of_block=mm_of_block, CS1=CS1, KC1=KC1,
        CC2=CC2, n_cores=n_cores,
    )
    return in_maps, meta


# ------------------------------------------------------------- device build


def build_nc(meta):
    N = meta["N"]
    T = meta["T"]
    C = meta["C"]
    NPC = meta["NPC"]
    NBLK = meta["NBLK"]
    NG = meta["NG"]
    NA = meta["NA"]
    NB = meta["NB"]
    NCH = meta["NCH"]
    NCHA = meta["NCHA"]
    NCHB = meta["NCHB"]
    NCHG = meta["NCHG"]
    NIND = meta["NIND"]
    ind_cols = meta["ind_cols"]
    mm_of_block = meta["mm_of_block"]
    CS1 = meta["CS1"]
    KC1 = meta["KC1"]
    CC2 = meta["CC2"]
    KC2 = len(CC2)
    n_cores = meta["n_cores"]
    rg = [list(range(n_cores))]

    nc = bacc.Bacc(
        "TRN2", target_bir_lowering=False, debug=False, num_devices=n_cores
    )

    xsd = nc.dram_tensor("xsd", [NBLK * CS1, KC1 * P], BF16, kind="ExternalInput")
    w1ct = nc.dram_tensor("w1ct", [KC1, CS1, C], BF16, kind="ExternalInput")
    w2c = nc.dram_tensor("w2c", [KC2, P, C], BF16, kind="ExternalInput")
    b1r = nc.dram_tensor("b1r", [1, C], BF16, kind="ExternalInput")
    b2r = nc.dram_tensor("b2r", [1, C], BF16, kind="ExternalInput")
    iot = nc.dram_tensor("iot", [P, P], BF16, kind="ExternalInput")
    dist = nc.dram_tensor("dist", [P, NBLK], F32, kind="ExternalInput")
    idxt = nc.dram_tensor("idxt", [P, NG * NCHG], I32, kind="ExternalInput")
    dlt = nc.dram_tensor("dlt", [P, NG * NIND], BF16, kind="ExternalInput")
    out_ext = nc.dram_tensor("out", [NPC, C], F32, kind="ExternalOutput")

    ACT = mybir.ActivationFunctionType

    with tile.TileContext(nc) as tc:
        with (
            tc.tile_pool(name="dramp", bufs=1, space="DRAM") as dramp,
            tc.tile_pool(name="singles", bufs=1) as singles,
            tc.tile_pool(name="xp", bufs=3) as xp,
            tc.tile_pool(name="sp", bufs=2) as sp,
            tc.tile_pool(name="wp", bufs=4) as wp,
            tc.tile_pool(name="hp", bufs=4) as hpp_pool,
            tc.tile_pool(name="msA", bufs=4) as mpA,
            tc.tile_pool(name="msB", bufs=2) as mpB,
            tc.tile_pool(name="inds", bufs=2) as ip,
            tc.tile_pool(name="psA", bufs=2, space="PSUM") as psA,
            tc.tile_pool(name="psT", bufs=2, space="PSUM") as psT,
            tc.tile_pool(name="psB", bufs=2, space="PSUM") as psB,
            tc.tile_pool(name="psC", bufs=2, space="PSUM") as psC,
        ):
            ag1A = dramp.tile([NA, C], BF16, name="ag1A")
            ag1B = dramp.tile([NB, C], BF16, name="ag1B")
            hp1A = dramp.tile([n_cores * NA, C], BF16, addr_space="Shared", name="hp1A")
            hp1B = dramp.tile([n_cores * NB, C], BF16, addr_space="Shared", name="hp1B")
            ag2A = dramp.tile([NA, C], BF16, name="ag2A")
            ag2B = dramp.tile([NB, C], BF16, name="ag2B")
            hp2A = dramp.tile([n_cores * NA, C], BF16, addr_space="Shared", name="hp2A")
            hp2B = dramp.tile([n_cores * NB, C], BF16, addr_space="Shared", name="hp2B")

            ident = singles.tile([P, P], BF16, name="ident")
            make_identity(nc, ident[:])
            ones1 = singles.tile([1, P], BF16, name="ones1")
            nc.vector.memset(ones1[:], 1.0)
            w1sb = singles.tile([CS1, KC1, C], BF16, name="w1sb")
            for k in range(KC1):
                nc.sync.dma_start(out=w1sb[:, k, :], in_=w1ct[k])
            w2sb = singles.tile([P, KC2, C], BF16, name="w2sb")
            for k in range(KC2):
                nc.sync.dma_start(out=w2sb[:, k, :], in_=w2c[k])
            b1sb = singles.tile([1, C], BF16, name="b1sb")
            nc.sync.dma_start(out=b1sb[:], in_=b1r[:])
            b2sb = singles.tile([1, C], BF16, name="b2sb")
            nc.sync.dma_start(out=b2sb[:], in_=b2r[:])
            iosb = singles.tile([P, 1, P], BF16, name="iosb")
            nc.sync.dma_start(out=iosb[:, 0, :], in_=iot[:])
            dissb = singles.tile([P, NBLK], F32, name="dissb")
            nc.sync.dma_start(out=dissb[:], in_=dist[:])
            idxsb = singles.tile([P, NG * NCHG], I32, name="idxsb")
            nc.sync.dma_start(out=idxsb[:], in_=idxt[:])
            dlsb = singles.tile([P, NG * NIND], BF16, name="dlsb")
            nc.sync.dma_start(out=dlsb[:], in_=dlt[:])

            hps1 = singles.tile([P, NBLK, C], BF16, name="hps1")
            hps2 = singles.tile([P, NBLK, C], BF16, name="hps2")

            def ag(agin, hpf):
                nc.gpsimd.collective_compute(
                    "AllGather",
                    mybir.AluOpType.bypass,
                    replica_groups=rg,
                    ins=[agin.opt()],
                    outs=[hpf.opt()],
                )

            # ---------------- stage A
            for b in range(NBLK):
                Pb = min(P, NPC - b * P)
                dcol = dissb[:, b : b + 1]
                xt = xp.tile([CS1, KC1, P], BF16, tag="xt")
                nc.sync.dma_start(out=xt[:], in_=xsd[b * CS1 : (b + 1) * CS1, :])
                hpt = psA.tile([P, C], F32, tag="hpt")
                for k in range(KC1):
                    nc.tensor.matmul(
                        out=hpt[:],
                        lhsT=xt[:, k, :],
                        rhs=w1sb[:, k, :],
                        start=(k == 0),
                        stop=False,
                    )
                nc.tensor.matmul(
                    out=hpt[:], lhsT=ones1[:], rhs=b1sb[:], start=False, stop=True
                )
                hp_t = wp.tile([P, C], BF16, tag="hp")
                nc.scalar.activation(out=hp_t[:], in_=hpt[:], func=ACT.Copy, scale=dcol)
                if b < ASPLIT:
                    nc.sync.dma_start(
                        out=ag1A[b * P : b * P + Pb], in_=hp_t[:Pb]
                    )
                else:
                    r0 = (b - ASPLIT) * P
                    nc.sync.dma_start(out=ag1B[r0 : r0 + Pb], in_=hp_t[:Pb])
                nc.scalar.activation(
                    out=hps1[:, b, :], in_=hp_t[:], func=ACT.Copy, scale=dcol
                )
                if b == ASPLIT - 1:
                    with tc.high_priority():
                        ag(ag1A, hp1A)
                if b == NBLK - 1:
                    with tc.high_priority():
                        ag(ag1B, hp1B)

            # ------------- shared prop helpers
            def emit_gathers(g, h, srcA, srcB, pool):
                ncols = int(NCH[g][h])
                nmax = NCHA if h == 0 else NCHB
                msg = pool.tile([P, nmax, C], BF16, tag=f"m{h}")
                src = srcA if h == 0 else srcB
                base = g * NCHG + (0 if h == 0 else NCHA)
                for j in range(ncols):
                    nc.gpsimd.indirect_dma_start(
                        out=msg[:, j, :],
                        out_offset=None,
                        in_=src[:],
                        in_offset=IndirectOffsetOnAxis(
                            ap=idxsb[:, base + j : base + j + 1], axis=0
                        ),
                    )
                return msg

            def emit_ind(g):
                ind = ip.tile([P, NIND, P], BF16, tag="ind")
                nc.vector.tensor_tensor(
                    out=ind[:],
                    in0=iosb[:, :, :].to_broadcast([P, NIND, P]),
                    in1=dlsb[:, g * NIND : (g + 1) * NIND].to_broadcast(
                        [P, NIND, P]
                    ),
                    op=mybir.AluOpType.is_equal,
                )
                return ind

            def prop_block(b, msgA, msgB, ind):
                mm = mm_of_block[b]
                pp = psB.tile([P, C], F32, tag="pp")
                for i, (icol, h, j) in enumerate(mm):
                    msg = msgA if h == 0 else msgB
                    nc.tensor.matmul(
                        out=pp[:],
                        lhsT=ind[:, icol, :],
                        rhs=msg[:, j, :],
                        start=(i == 0),
                        stop=(i == len(mm) - 1),
                    )
                return pp

            # schedule tokens for a prop pass: stagger A-gathers one group
            # ahead of B-gathers so the pool queue rarely stalls on segment B
            def prop_pass(srcA, srcB, consume):
                msgsA = {}
                msgsB = {}
                inds = {}
                for gg in range(min(3, NG)):
                    msgsA[gg] = emit_gathers(gg, 0, srcA, srcB, mpA)
                    inds[gg] = emit_ind(gg)
                for g in range(NG):
                    msgsB[g] = emit_gathers(g, 1, srcA, srcB, mpB)
                    if g + 3 < NG:
                        msgsA[g + 3] = emit_gathers(g + 3, 0, srcA, srcB, mpA)
                        inds[g + 3] = emit_ind(g + 3)
                    for bl in range(GB):
                        b = g * GB + bl
                        consume(b, msgsA[g], msgsB[g], inds[g])
                    del msgsA[g], msgsB[g], inds[g]

            # ---------------- layer 1 prop + layer 2 linear
            def consume_l1(b, msgA, msgB, ind):
                Pb = min(P, NPC - b * P)
                dcol = dissb[:, b : b + 1]
                pp = prop_block(b, msgA, msgB, ind)
                t1 = hpp_pool.tile([P, C], F32, tag="t1")
                nc.vector.scalar_tensor_tensor(
                    out=t1[:],
                    in0=pp[:],
                    scalar=dcol,
                    in1=hps1[:, b, :],
                    op0=mybir.AluOpType.mult,
                    op1=mybir.AluOpType.add,
                )
                h1 = hpp_pool.tile([P, C], BF16, tag="h1")
                nc.vector.scalar_tensor_tensor(
                    out=h1[:],
                    in0=t1[:],
                    scalar=0.01,
                    in1=t1[:],
                    op0=mybir.AluOpType.mult,
                    op1=mybir.AluOpType.max,
                )
                h2p = psC.tile([P, C], F32, tag="h2p")
                for k, (c0, cs) in enumerate(CC2):
                    ptr2 = psT.tile([P, P], BF16, tag="ptr")
                    nc.tensor.transpose(
                        out=ptr2[:cs, :], in_=h1[:, c0 : c0 + cs], identity=ident[:]
                    )
                    hT = wp.tile([P, P], BF16, tag="hT")
                    nc.vector.tensor_copy(out=hT[:cs, :], in_=ptr2[:cs, :])
                    nc.tensor.matmul(
                        out=h2p[:],
                        lhsT=hT[:cs, :],
                        rhs=w2sb[:cs, k, :],
                        start=(k == 0),
                        stop=False,
                    )
                nc.tensor.matmul(
                    out=h2p[:], lhsT=ones1[:], rhs=b2sb[:], start=False, stop=True
                )
                hp2_t = wp.tile([P, C], BF16, tag="hp2")
                nc.scalar.activation(
                    out=hp2_t[:], in_=h2p[:], func=ACT.Copy, scale=dcol
                )
                if b < ASPLIT:
                    nc.sync.dma_start(out=ag2A[b * P : b * P + Pb], in_=hp2_t[:Pb])
                else:
                    r0 = (b - ASPLIT) * P
                    nc.sync.dma_start(out=ag2B[r0 : r0 + Pb], in_=hp2_t[:Pb])
                nc.scalar.activation(
                    out=hps2[:, b, :], in_=hp2_t[:], func=ACT.Copy, scale=dcol
                )
                # trigger the layer-2 AllGather segments as soon as their
                # inputs are complete (two blocks of slack for the epilogue)
                if b == ASPLIT + 1:
                    ag(ag2A, hp2A)
                if b == NBLK - 1:
                    ag(ag2B, hp2B)

            prop_pass(hp1A, hp1B, consume_l1)

            # ---------------- layer 2 prop -> output
            def consume_l2(b, msgA, msgB, ind):
                Pb = min(P, NPC - b * P)
                dcol = dissb[:, b : b + 1]
                pp = prop_block(b, msgA, msgB, ind)
                ot = hpp_pool.tile([P, C], F32, tag="ot")
                nc.vector.scalar_tensor_tensor(
                    out=ot[:],
                    in0=pp[:],
                    scalar=dcol,
                    in1=hps2[:, b, :],
                    op0=mybir.AluOpType.mult,
                    op1=mybir.AluOpType.add,
                )
                nc.sync.dma_start(out=out_ext[b * P : b * P + Pb], in_=ot[:Pb])

            prop_pass(hp2A, hp2B, consume_l2)

    nc.compile()
    return nc


# ------------------------------------------------------------------ runner

_CACHE = {}


def run(x, edge_index, W1, b1, W2, b2, n_cores=N_CORES, trace=False):
    in_maps, meta = prep_inputs(x, edge_index, W1, b1, W2, b2, n_cores)
    key = (meta["N"], meta["T"], meta["C"], meta["NCHA"], meta["NCHB"],
           meta["NIND"], n_cores)
    if key not in _CACHE:
        _CACHE[key] = build_nc(meta)
    nc = _CACHE[key]
    res = bass_utils.run_bass_kernel_spmd(
        nc, in_maps, core_ids=list(range(n_cores)), trace=trace
    )
    outs = [np.asarray(res.results[c]["out"]) for c in range(n_cores)]
    full = np.concatenate(outs, axis=0).astype(np.float32)
    return full, res


def kernel(x, edge_index, W1, b1, W2, b2):
    x = np.asarray(x)
    edge_index = np.asarray(edge_index)
    full, _ = run(
        np.asarray(x, np.float32),
        edge_index,
        np.asarray(W1, np.float32),
        np.asarray(b1, np.float32),
        np.asarray(W2, np.float32),
        np.asarray(b2, np.float32),
    )
    return full


# revision 6
# speedup vs baseline: 1.2393x; 1.0229x over previous
"""GCN (2-layer, symmetric-norm message passing) on 8 Trainium2 NeuronCores.

v3: destination-sharded GCN with
  - per-chunk [P,1] indirect-DMA gathers (the only HW-correct gather form),
  - group-pooled edge chunks (7 dst blocks per group, chunks packed across
    block boundaries) to minimize the ~1us/call SWDGE fixed cost,
  - the feature table / AllGather split into two segments (blocks 0..23 /
    24..48 of each core) so each AllGather overlaps neighboring compute,
  - host-pretransposed x so stage A needs no on-chip transposes,
  - biases added via rank-1 ones-matmuls into PSUM,
  - batched indicator construction (one 3D is_equal per group).
"""

import numpy as np

import concourse.bacc as bacc
import concourse.bass as bass
import concourse.tile as tile
from concourse import bass_utils, mybir
from concourse.bass import IndirectOffsetOnAxis
from concourse.masks import make_identity

F32 = mybir.dt.float32
BF16 = mybir.dt.bfloat16
I32 = mybir.dt.int32
P = 128

N_CORES = 8
GB = 7          # dst blocks per gather group
ASPLIT = 24     # table segment split (blocks 0..ASPLIT-1 -> A, rest -> B)


def _cdiv(a, b):
    return (a + b - 1) // b


# ---------------------------------------------------------------- host prep


def prep_inputs(x, edge_index, W1, b1, W2, b2, n_cores=N_CORES):
    import ml_dtypes

    N, T, C = x.shape
    NPC = N // n_cores
    NBLK = _cdiv(NPC, P)
    NG = NBLK // GB
    NA = ASPLIT * P            # rows per core in segment A
    NB = NPC - NA              # rows per core in segment B

    row = np.asarray(edge_index[0], dtype=np.int64)
    col = np.asarray(edge_index[1], dtype=np.int64)

    deg = (np.bincount(row, minlength=N) + 1).astype(np.float32)
    dis = (deg.astype(np.float32) ** -0.5).astype(np.float32)

    # table-row mapping for source node g: half, row
    src_r = row // NPC
    src_i = row % NPC
    in_A = src_i < NA
    tab_row = np.where(in_A, src_r * NA + src_i, src_r * NB + (src_i - NA))

    core_of = col // NPC

    # per-core, per-group, per-half edge lists (sorted by dst)
    # lists[c][g][h] = (tab_rows, dst_local_in_block, dst_block)
    lists = [[[None, None] for _ in range(NG)] for _ in range(n_cores)]
    for c in range(n_cores):
        m = core_of == c
        tr = tab_row[m]
        ia = in_A[m]
        d = col[m] - c * NPC
        order = np.argsort(d, kind="stable")
        tr, ia, d = tr[order], ia[order], d[order]
        blk = d // P
        grp = blk // GB
        for g in range(NG):
            gm = grp == g
            for h in range(2):
                hm = gm & (ia == (h == 0))
                lists[c][g][h] = (
                    tr[hm].astype(np.int64),
                    (d[hm] - blk[hm] * P).astype(np.int64),
                    blk[hm].astype(np.int64),
                )

    # chunk counts per (g, half): max over cores
    NCH = np.zeros((NG, 2), np.int64)
    for g in range(NG):
        for h in range(2):
            NCH[g, h] = max(
                _cdiv(len(lists[c][g][h][0]), P) for c in range(n_cores)
            )
    NCHA = int(NCH[:, 0].max())
    NCHB = int(NCH[:, 1].max())
    NCHG = NCHA + NCHB  # idx cols per group (uniform layout)

    # indicator columns: union over cores of blocks spanned by each chunk
    # ind_cols[g] = list of (h, jloc, b); mm_of_block[b] = ordered list of
    # (ind_local, h, jloc)
    ind_cols = []
    mm_of_block = [[] for _ in range(NBLK)]
    for g in range(NG):
        cols = []
        for h in range(2):
            for j in range(int(NCH[g, h])):
                spanned = set()
                for c in range(n_cores):
                    blks = lists[c][g][h][2][j * P : (j + 1) * P]
                    spanned.update(np.unique(blks).tolist())
                for b in sorted(spanned):
                    cols.append((h, j, int(b)))
        ind_cols.append(cols)
        for i, (h, j, b) in enumerate(cols):
            mm_of_block[b].append((i, h, j))
    NIND = max(len(c) for c in ind_cols)

    # replicated weights; stage-A contraction runs over (c,t) jointly so the
    # T-mean folds into the matmul: 1200 rows split into KC1 chunks of CS1.
    CS1 = 120
    KC1 = (C * T) // CS1
    w1rows = np.repeat(W1.T, T, axis=0) / np.float32(T)  # row c*T+t -> W1.T[c]/T
    w1ct = np.zeros((KC1, CS1, C), ml_dtypes.bfloat16)
    for k in range(KC1):
        w1ct[k] = w1rows[k * CS1 : (k + 1) * CS1, :].astype(np.float32)
    CC2 = [(c0, min(P, C - c0)) for c0 in range(0, C, P)]
    KC2 = len(CC2)
    w2c = np.zeros((KC2, P, C), ml_dtypes.bfloat16)
    for k, (c0, cs) in enumerate(CC2):
        w2c[k, :cs, :] = W2.T[c0 : c0 + cs, :].astype(np.float32)
    b1r = np.asarray(b1, np.float32).reshape(1, C).astype(ml_dtypes.bfloat16)
    b2r = np.asarray(b2, np.float32).reshape(1, C).astype(ml_dtypes.bfloat16)
    iot = np.broadcast_to(np.arange(P, dtype=np.float32), (P, P)).astype(
        ml_dtypes.bfloat16
    )

    in_maps = []
    for c in range(n_cores):
        idxt = np.zeros((P, NG * NCHG), np.int32)
        dlt = np.full((P, NG * NIND), -1.0, np.float32)
        for g in range(NG):
            for h in range(2):
                tr, dl, blk = lists[c][g][h]
                n = len(tr)
                nch = int(NCH[g, h])
                pad = nch * P
                trp = np.zeros(pad, np.int64)
                trp[:n] = tr
                base = g * NCHG + (0 if h == 0 else NCHA)
                idxt[:, base : base + nch] = trp.reshape(nch, P).T.astype(
                    np.int32
                )
            for i, (h, j, b) in enumerate(ind_cols[g]):
                tr, dl, blk = lists[c][g][h]
                n = len(tr)
                sl = np.full(P, -1.0, np.float32)
                s0 = j * P
                seg = slice(s0, min(s0 + P, n))
                nseg = seg.stop - seg.start
                if nseg > 0:
                    dseg = dl[seg]
                    bseg = blk[seg]
                    vals = np.where(bseg == b, dseg.astype(np.float32), -1.0)
                    sl[:nseg] = vals
                dlt[:, g * NIND + i] = sl

        dis_c = dis[c * NPC : (c + 1) * NPC]
        flat = np.zeros(NBLK * P, np.float32)
        flat[:NPC] = dis_c
        dist = flat.reshape(NBLK, P).T.copy()

        xc = np.asarray(x[c * NPC : (c + 1) * NPC], np.float32)
        xpad = np.zeros((NBLK * P, T, C), np.float32)
        xpad[:NPC] = xc
        # [NBLK, P(n), T, C] -> rows j=c*T+t chunked [NBLK, KC1, CS1, P] ->
        # per-block tile [CS1, KC1, P]
        xr = xpad.reshape(NBLK, P, T, C).transpose(0, 3, 2, 1)  # [NBLK, C, T, P]
        xr = xr.reshape(NBLK, KC1, CS1, P).transpose(0, 2, 1, 3)
        xsd = (
            np.ascontiguousarray(xr)
            .reshape(NBLK * CS1, KC1 * P)
            .astype(ml_dtypes.bfloat16)
        )

        in_maps.append(
            {
                "xsd": xsd,
                "w1ct": w1ct,
                "w2c": w2c,
                "b1r": b1r,
                "b2r": b2r,
                "iot": iot,
                "dist": dist,
                "idxt": idxt,
                "dlt": dlt.astype(ml_dtypes.bfloat16),
            }
        )

    meta = dict(
        N=N, T=T, C=C, NPC=NPC, NBLK=NBLK, NG=NG, NA=NA, NB=NB,
        NCH=NCH.tolist(), NCHA=NCHA, NCHB=NCHB, NCHG=NCHG, NIND=NIND,
        ind_cols=ind_cols, mm_of_block=mm_of_block, CS1=CS1, KC1=KC1,
        CC2=CC2, n_cores=n_cores,
    )
    return in_maps, meta


# ------------------------------------------------------------- device build


def build_nc(meta):
    N = meta["N"]
    T = meta["T"]
    C = meta["C"]
    NPC = meta["NPC"]
    NBLK = meta["NBLK"]
    NG = meta["NG"]
    NA = meta["NA"]
    NB = meta["NB"]
    NCH = meta["NCH"]
    NCHA = meta["NCHA"]
    NCHB = meta["NCHB"]
    NCHG = meta["NCHG"]
    NIND = meta["NIND"]
    ind_cols = meta["ind_cols"]
    mm_of_block = meta["mm_of_block"]
    CS1 = meta["CS1"]
    KC1 = meta["KC1"]
    CC2 = meta["CC2"]
    KC2 = len(CC2)
    n_cores = meta["n_cores"]
    rg = [list(range(n_cores))]

    nc = bacc.Bacc(
        "TRN2", target_bir_lowering=False, debug=False, num_devices=n_cores
    )

    xsd = nc.dram_tensor("xsd", [NBLK * CS1, KC1 * P], BF16, kind="ExternalInput")
    w1ct = nc.dram_tensor("w1ct", [KC1, CS1, C], BF16, kind="ExternalInput")
    w2c = nc.dram_tensor("w2c", [KC2, P, C], BF16, kind="ExternalInput")
    b1r = nc.dram_tensor("b1r", [1, C], BF16, kind="ExternalInput")
    b2r = nc.dram_tensor("b2r", [1, C], BF16, kind="ExternalInput")
    iot = nc.dram_tensor("iot", [P, P], BF16, kind="ExternalInput")
    dist = nc.dram_tensor("dist", [P, NBLK], F32, kind="ExternalInput")
    idxt = nc.dram_tensor("idxt", [P, NG * NCHG], I32, kind="ExternalInput")
    dlt = nc.dram_tensor("dlt", [P, NG * NIND], BF16, kind="ExternalInput")
    out_ext = nc.dram_tensor("out", [NPC, C], F32, kind="ExternalOutput")

    ACT = mybir.ActivationFunctionType

    with tile.TileContext(nc) as tc:
        with (
            tc.tile_pool(name="dramp", bufs=1, space="DRAM") as dramp,
            tc.tile_pool(name="dr1a", bufs=1, space="DRAM") as dr1a,
            tc.tile_pool(name="dr1b", bufs=1, space="DRAM") as dr1b,
            tc.tile_pool(name="dr2a", bufs=1, space="DRAM") as dr2a,
            tc.tile_pool(name="dr2b", bufs=1, space="DRAM") as dr2b,
            tc.tile_pool(name="singles", bufs=1) as singles,
            tc.tile_pool(name="xp", bufs=3) as xp,
            tc.tile_pool(name="sp", bufs=2) as sp,
            tc.tile_pool(name="wp", bufs=4) as wp,
            tc.tile_pool(name="hp", bufs=4) as hpp_pool,
            tc.tile_pool(name="msA", bufs=4) as mpA,
            tc.tile_pool(name="msB", bufs=2) as mpB,
            tc.tile_pool(name="inds", bufs=2) as ip,
            tc.tile_pool(name="psA", bufs=2, space="PSUM") as psA,
            tc.tile_pool(name="psT", bufs=2, space="PSUM") as psT,
            tc.tile_pool(name="psB", bufs=2, space="PSUM") as psB,
            tc.tile_pool(name="psC", bufs=2, space="PSUM") as psC,
        ):
            ag1A = dr1a.tile([NA, C], BF16, name="ag1A")
            ag1B = dr1b.tile([NB, C], BF16, name="ag1B")
            hp1A = dr1a.tile([n_cores * NA, C], BF16, addr_space="Shared", name="hp1A")
            hp1B = dr1b.tile([n_cores * NB, C], BF16, addr_space="Shared", name="hp1B")
            ag2A = dr2a.tile([NA, C], BF16, name="ag2A")
            ag2B = dr2b.tile([NB, C], BF16, name="ag2B")
            hp2A = dr2a.tile([n_cores * NA, C], BF16, addr_space="Shared", name="hp2A")
            hp2B = dr2b.tile([n_cores * NB, C], BF16, addr_space="Shared", name="hp2B")

            ident = singles.tile([P, P], BF16, name="ident")
            make_identity(nc, ident[:])
            ones1 = singles.tile([1, P], BF16, name="ones1")
            nc.vector.memset(ones1[:], 1.0)
            w1sb = singles.tile([CS1, KC1, C], BF16, name="w1sb")
            for k in range(KC1):
                nc.sync.dma_start(out=w1sb[:, k, :], in_=w1ct[k])
            w2sb = singles.tile([P, KC2, C], BF16, name="w2sb")
            for k in range(KC2):
                nc.sync.dma_start(out=w2sb[:, k, :], in_=w2c[k])
            b1sb = singles.tile([1, C], BF16, name="b1sb")
            nc.sync.dma_start(out=b1sb[:], in_=b1r[:])
            b2sb = singles.tile([1, C], BF16, name="b2sb")
            nc.sync.dma_start(out=b2sb[:], in_=b2r[:])
            iosb = singles.tile([P, 1, P], BF16, name="iosb")
            nc.sync.dma_start(out=iosb[:, 0, :], in_=iot[:])
            dissb = singles.tile([P, NBLK], F32, name="dissb")
            nc.sync.dma_start(out=dissb[:], in_=dist[:])
            idxsb = singles.tile([P, NG * NCHG], I32, name="idxsb")
            nc.sync.dma_start(out=idxsb[:], in_=idxt[:])
            dlsb = singles.tile([P, NG * NIND], BF16, name="dlsb")
            nc.sync.dma_start(out=dlsb[:], in_=dlt[:])

            hps1 = singles.tile([P, NBLK, C], BF16, name="hps1")
            hps2 = singles.tile([P, NBLK, C], BF16, name="hps2")

            def ag(agin, hpf):
                nc.gpsimd.collective_compute(
                    "AllGather",
                    mybir.AluOpType.bypass,
                    replica_groups=rg,
                    ins=[agin.opt()],
                    outs=[hpf.opt()],
                )

            # ---------------- stage A
            for b in range(NBLK):
                Pb = min(P, NPC - b * P)
                dcol = dissb[:, b : b + 1]
                xt = xp.tile([CS1, KC1, P], BF16, tag="xt")
                nc.sync.dma_start(out=xt[:], in_=xsd[b * CS1 : (b + 1) * CS1, :])
                hpt = psA.tile([P, C], F32, tag="hpt")
                for k in range(KC1):
                    nc.tensor.matmul(
                        out=hpt[:],
                        lhsT=xt[:, k, :],
                        rhs=w1sb[:, k, :],
                        start=(k == 0),
                        stop=False,
                    )
                nc.tensor.matmul(
                    out=hpt[:], lhsT=ones1[:], rhs=b1sb[:], start=False, stop=True
                )
                hp_t = wp.tile([P, C], BF16, tag="hp")
                nc.scalar.activation(out=hp_t[:], in_=hpt[:], func=ACT.Copy, scale=dcol)
                if b < ASPLIT:
                    nc.sync.dma_start(
                        out=ag1A[b * P : b * P + Pb], in_=hp_t[:Pb]
                    )
                else:
                    r0 = (b - ASPLIT) * P
                    nc.sync.dma_start(out=ag1B[r0 : r0 + Pb], in_=hp_t[:Pb])
                nc.scalar.activation(
                    out=hps1[:, b, :], in_=hp_t[:], func=ACT.Copy, scale=dcol
                )
                if b == ASPLIT - 1:
                    with tc.high_priority():
                        ag(ag1A, hp1A)
                if b == NBLK - 1:
                    with tc.high_priority():
                        ag(ag1B, hp1B)

            # ------------- shared prop helpers
            def emit_gathers(g, h, srcA, srcB, pool):
                ncols = int(NCH[g][h])
                nmax = NCHA if h == 0 else NCHB
                msg = pool.tile([P, nmax, C], BF16, tag=f"m{h}")
                src = srcA if h == 0 else srcB
                base = g * NCHG + (0 if h == 0 else NCHA)
                for j in range(ncols):
                    nc.gpsimd.indirect_dma_start(
                        out=msg[:, j, :],
                        out_offset=None,
                        in_=src[:],
                        in_offset=IndirectOffsetOnAxis(
                            ap=idxsb[:, base + j : base + j + 1], axis=0
                        ),
                    )
                return msg

            def emit_ind(g):
                ind = ip.tile([P, NIND, P], BF16, tag="ind")
                nc.vector.tensor_tensor(
                    out=ind[:],
                    in0=iosb[:, :, :].to_broadcast([P, NIND, P]),
                    in1=dlsb[:, g * NIND : (g + 1) * NIND].to_broadcast(
                        [P, NIND, P]
                    ),
                    op=mybir.AluOpType.is_equal,
                )
                return ind

            def prop_block(b, msgA, msgB, ind):
                mm = mm_of_block[b]
                pp = psB.tile([P, C], F32, tag="pp")
                for i, (icol, h, j) in enumerate(mm):
                    msg = msgA if h == 0 else msgB
                    nc.tensor.matmul(
                        out=pp[:],
                        lhsT=ind[:, icol, :],
                        rhs=msg[:, j, :],
                        start=(i == 0),
                        stop=(i == len(mm) - 1),
                    )
                return pp

            # schedule tokens for a prop pass: stagger A-gathers one group
            # ahead of B-gathers so the pool queue rarely stalls on segment B
            def prop_pass(srcA, srcB, consume):
                msgsA = {}
                msgsB = {}
                inds = {}
                for gg in range(min(3, NG)):
                    msgsA[gg] = emit_gathers(gg, 0, srcA, srcB, mpA)
                    inds[gg] = emit_ind(gg)
                for g in range(NG):
                    msgsB[g] = emit_gathers(g, 1, srcA, srcB, mpB)
                    if g + 3 < NG:
                        msgsA[g + 3] = emit_gathers(g + 3, 0, srcA, srcB, mpA)
                        inds[g + 3] = emit_ind(g + 3)
                    for bl in range(GB):
                        b = g * GB + bl
                        consume(b, msgsA[g], msgsB[g], inds[g])
                    del msgsA[g], msgsB[g], inds[g]

            # ---------------- layer 1 prop + layer 2 linear
            def consume_l1(b, msgA, msgB, ind):
                Pb = min(P, NPC - b * P)
                dcol = dissb[:, b : b + 1]
                pp = prop_block(b, msgA, msgB, ind)
                t1 = hpp_pool.tile([P, C], F32, tag="t1")
                nc.vector.scalar_tensor_tensor(
                    out=t1[:],
                    in0=pp[:],
                    scalar=dcol,
                    in1=hps1[:, b, :],
                    op0=mybir.AluOpType.mult,
                    op1=mybir.AluOpType.add,
                )
                h1 = hpp_pool.tile([P, C], BF16, tag="h1")
                nc.vector.scalar_tensor_tensor(
                    out=h1[:],
                    in0=t1[:],
                    scalar=0.01,
                    in1=t1[:],
                    op0=mybir.AluOpType.mult,
                    op1=mybir.AluOpType.max,
                )
                h2p = psC.tile([P, C], F32, tag="h2p")
                for k, (c0, cs) in enumerate(CC2):
                    ptr2 = psT.tile([P, P], BF16, tag="ptr")
                    nc.tensor.transpose(
                        out=ptr2[:cs, :], in_=h1[:, c0 : c0 + cs], identity=ident[:]
                    )
                    hT = wp.tile([P, P], BF16, tag="hT")
                    nc.vector.tensor_copy(out=hT[:cs, :], in_=ptr2[:cs, :])
                    nc.tensor.matmul(
                        out=h2p[:],
                        lhsT=hT[:cs, :],
                        rhs=w2sb[:cs, k, :],
                        start=(k == 0),
                        stop=False,
                    )
                nc.tensor.matmul(
                    out=h2p[:], lhsT=ones1[:], rhs=b2sb[:], start=False, stop=True
                )
                hp2_t = wp.tile([P, C], BF16, tag="hp2")
                nc.scalar.activation(
                    out=hp2_t[:], in_=h2p[:], func=ACT.Copy, scale=dcol
                )
                if b < ASPLIT:
                    nc.sync.dma_start(out=ag2A[b * P : b * P + Pb], in_=hp2_t[:Pb])
                else:
                    r0 = (b - ASPLIT) * P
                    nc.sync.dma_start(out=ag2B[r0 : r0 + Pb], in_=hp2_t[:Pb])
                nc.scalar.activation(
                    out=hps2[:, b, :], in_=hp2_t[:], func=ACT.Copy, scale=dcol
                )
                # trigger the layer-2 AllGather segments as soon as their
                # inputs are complete (two blocks of slack for the epilogue)
                if b == ASPLIT + 1:
                    ag(ag2A, hp2A)
                if b == NBLK - 1:
                    ag(ag2B, hp2B)

            prop_pass(hp1A, hp1B, consume_l1)

            # ---------------- layer 2 prop -> output
            def consume_l2(b, msgA, msgB, ind):
                Pb = min(P, NPC - b * P)
                dcol = dissb[:, b : b + 1]
                pp = prop_block(b, msgA, msgB, ind)
                ot = hpp_pool.tile([P, C], F32, tag="ot")
                nc.vector.scalar_tensor_tensor(
                    out=ot[:],
                    in0=pp[:],
                    scalar=dcol,
                    in1=hps2[:, b, :],
                    op0=mybir.AluOpType.mult,
                    op1=mybir.AluOpType.add,
                )
                nc.sync.dma_start(out=out_ext[b * P : b * P + Pb], in_=ot[:Pb])

            prop_pass(hp2A, hp2B, consume_l2)

    nc.compile()
    return nc


# ------------------------------------------------------------------ runner

_CACHE = {}


def run(x, edge_index, W1, b1, W2, b2, n_cores=N_CORES, trace=False):
    in_maps, meta = prep_inputs(x, edge_index, W1, b1, W2, b2, n_cores)
    key = (meta["N"], meta["T"], meta["C"], meta["NCHA"], meta["NCHB"],
           meta["NIND"], n_cores)
    if key not in _CACHE:
        _CACHE[key] = build_nc(meta)
    nc = _CACHE[key]
    res = bass_utils.run_bass_kernel_spmd(
        nc, in_maps, core_ids=list(range(n_cores)), trace=trace
    )
    outs = [np.asarray(res.results[c]["out"]) for c in range(n_cores)]
    full = np.concatenate(outs, axis=0).astype(np.float32)
    return full, res


def kernel(x, edge_index, W1, b1, W2, b2):
    x = np.asarray(x)
    edge_index = np.asarray(edge_index)
    full, _ = run(
        np.asarray(x, np.float32),
        edge_index,
        np.asarray(W1, np.float32),
        np.asarray(b1, np.float32),
        np.asarray(W2, np.float32),
        np.asarray(b2, np.float32),
    )
    return full
